# revision 1
# baseline (speedup 1.0000x reference)
"""Bipolar morphological conv2d kernel for Trainium2 (8 NeuronCores).

Math: reference computes, per output position and out-channel c,
    y = m(lp1,K1) - m(lp1,K2) - m(lp2,K1) + m(lp2,K2) + bias
with m(logp, k)[c] = exp(max_p(logp_p + k_pc)), lp1 = log(max(patch, .1)),
lp2 = log(max(-patch, .1)).

Since exp is monotone, exp(max_p(log(max(x,.1)) + k)) = max_p(max(x,.1)*K_pc)
with K = exp(k) > 0.  Further, the clamp folds into a per-channel constant:
    max_p(max(x_p,.1)*K_pc) = max(U_c, max_p(x_p*K_pc)),  U_c = .1*max_p K_pc
(because entries with x_p <= .1 contribute x_p*K <= .1*K <= U_c, and the true
value is always >= U_c).  Likewise the "-x" side is max(U_c, max_p(-x_p*K_pc)).
So the whole op is 4 max-times reductions over unclamped products x_p*K_pc.

Device strategy (data-parallel, one batch image per core):
  - partitions = 128 = [64 out-channels "A side" (+x) | 64 out-channels "B side" (-x)]
  - free dim   = 900 output positions, addressed as [30 rows, 30 cols] windows
    (row stride 32) into the pixel-linear broadcast row
  - x row per input channel is broadcast across partitions as [+x;...;-x;...]
    via a K=1 PE matmul (lhsT = [+1]*64+[-1]*64) into PSUM, staged to SBUF by
    the Scalar engine.
  - per (tap, ci) one fused scalar_tensor_tensor (mult then max) per kernel
    accumulator: acc_k = max(acc_k, xwin * K_k[(tap,ci), c])  -- 576 DVE ops,
    which is the roofline: DVE f32 3-src ops run at 1 elem/cycle/lane.
  - combine: one accumulating PE matmul pair per 128-position chunk computes
    (accA1-accB1)-(accA2-accB2) transposed to position-major; add bias; DMA.
Host precomputes exp(k), U_c, the packed per-partition scalar tables, and the
transposed/padded x rows.
"""

import os
from contextlib import ExitStack

import numpy as np

import concourse.bass as bass
import concourse.mybir as mybir
from concourse import bacc
import concourse.tile as tile
from concourse.bass_utils import run_bass_kernel_spmd

N_CORES = 8
H = W = C = 32
COUT = 64
HO = WO = 30
NPIX = H * W          # 1024
FD = HO * WO          # 900 output positions, accessed as [30, 30] windows
XLEN = 1026           # broadcast-row length: max tap offset 66 + 30*32 window
P = 288               # 3*3*32 patch size

F32 = mybir.dt.float32
F16 = mybir.dt.float16
_cache: dict = {}
last_results = None


def _ensure_axon_ntff_hook():
    """The trimmed agent image lacks antenv.axon_hooks; recreate it so
    run_bass_kernel_spmd(trace=True) can capture NTFF profiles. No-op on
    failure (tracing then just degrades)."""
    import sys
    import types

    try:
        import antenv.axon_hooks  # noqa: F401
        return
    except ImportError:
        pass
    try:
        mod = types.ModuleType("antenv.axon_hooks")
        holder = [None]
        mod.set_axon_ntff_profile_hook = lambda h: holder.__setitem__(0, h)
        mod.get_axon_ntff_profile_hook = lambda: holder[0]
        sys.modules["antenv.axon_hooks"] = mod
        from trn_agent_boot.trn_boot import _ntff_profile_via_ctypes

        so = "/opt/axon/libaxon_pjrt.so"
        if os.path.exists(so):
            holder[0] = _ntff_profile_via_ctypes(so)
    except Exception:
        pass


def _build_module():
    nc = bacc.Bacc()
    Alu = mybir.AluOpType

    xT = nc.dram_tensor("xT", [1, C * XLEN], F32, kind="ExternalInput")
    S1 = nc.dram_tensor("S1", [128, P], F32, kind="ExternalInput")
    S2 = nc.dram_tensor("S2", [128, P], F32, kind="ExternalInput")
    UB = nc.dram_tensor("UB", [128, 2], F32, kind="ExternalInput")
    BC = nc.dram_tensor("BC", [128, COUT], F32, kind="ExternalInput")
    PM = nc.dram_tensor("PM", [1, 128], F32, kind="ExternalInput")
    M1 = nc.dram_tensor("M1", [128, COUT], F16, kind="ExternalInput")
    M2 = nc.dram_tensor("M2", [128, COUT], F16, kind="ExternalInput")
    Y = nc.dram_tensor("Y", [HO * WO, COUT], F32, kind="ExternalOutput")

    with tile.TileContext(nc) as tc, ExitStack() as ctx:
        const = ctx.enter_context(tc.tile_pool(name="const", bufs=1))
        xbp = ctx.enter_context(tc.tile_pool(name="xbp", bufs=2, space="PSUM"))
        xbs = ctx.enter_context(tc.tile_pool(name="xbs", bufs=3))
        accp = ctx.enter_context(tc.tile_pool(name="accp", bufs=1))
        prodp = ctx.enter_context(tc.tile_pool(name="prodp", bufs=4))
        tps = ctx.enter_context(tc.tile_pool(name="tps", bufs=2, space="PSUM"))
        tsb = ctx.enter_context(tc.tile_pool(name="tsb", bufs=2))

        xT_sb = const.tile([1, C * XLEN], F32)
        nc.gpsimd.dma_start(out=xT_sb[:, :], in_=xT[:, :])
        S1_sb = const.tile([128, P], F32)
        nc.gpsimd.dma_start(out=S1_sb[:, :], in_=S1[:, :])
        S2_sb = const.tile([128, P], F32)
        nc.gpsimd.dma_start(out=S2_sb[:, :], in_=S2[:, :])
        UB_sb = const.tile([128, 2], F32)
        nc.gpsimd.dma_start(out=UB_sb[:, :], in_=UB[:, :])
        BC_sb = const.tile([128, COUT], F32)
        nc.gpsimd.dma_start(out=BC_sb[:, :], in_=BC[:, :])
        PM_sb = const.tile([1, 128], F32)
        nc.gpsimd.dma_start(out=PM_sb[:, :], in_=PM[:, :])
        M1_sb = const.tile([128, COUT], F16)
        nc.gpsimd.dma_start(out=M1_sb[:, :], in_=M1[:, :])
        M2_sb = const.tile([128, COUT], F16)
        nc.gpsimd.dma_start(out=M2_sb[:, :], in_=M2[:, :])

        # accW = two independent copies of [K1 | K2] accs side by side, fp16;
        # partitions = [A(+x)|B(-x)].  One TT folds TWO iterations' products.
        accW = accp.tile([128, 4 * FD], F16)
        nc.gpsimd.memset(accW[:, :], 0.0)
        for h in range(4):
            nc.vector.tensor_scalar(
                out=accW[:, h * FD : (h + 1) * FD],
                in0=accW[:, h * FD : (h + 1) * FD],
                scalar1=UB_sb[:, h % 2 : h % 2 + 1], scalar2=None, op0=Alu.add,
            )
        pending = []  # software pipeline: fold product pairs one TT late
        pp = None

        for ci in range(C):
            # broadcast row ci of xT to [ +x (64 parts) ; -x (64 parts) ]
            xq = xbp.tile([128, XLEN], F32)
            for s, e in ((0, 512), (512, 1024), (1024, XLEN)):
                nc.tensor.matmul(
                    xq[:, s:e], lhsT=PM_sb[:, :], rhs=xT_sb[0:1, ci * XLEN + s : ci * XLEN + e],
                    start=True, stop=True,
                )
            # fp16 staging, two parities so every tap window is 4B-aligned
            xbE = xbs.tile([128, XLEN], F16, tag="xbE")
            nc.scalar.copy(out=xbE[:, :], in_=xq[:, :])
            xbO = xbs.tile([128, XLEN - 1], F16, tag="xbO")
            nc.scalar.copy(out=xbO[:, :], in_=xq[:, 1:XLEN])

            for t in range(9):
                i, j = divmod(t, 3)
                off = i * W + j
                p = t * C + ci
                # 30x30 output window at tap offset, row stride W (even base)
                if off % 2 == 0:
                    src = xbE[:, off : off + HO * W]
                else:
                    src = xbO[:, off - 1 : off - 1 + HO * W]
                in0 = src.rearrange("q (a b) -> q a b", b=W)[:, :, :WO]
                k = ci * 9 + t
                if k % 2 == 0:
                    pp = prodp.tile([128, 4 * FD], F16)
                base = (k % 2) * 2 * FD
                for lo, S_sb in ((0, S1_sb), (FD, S2_sb)):
                    nc.vector.tensor_scalar(
                        out=pp[:, base + lo : base + lo + FD].rearrange(
                            "q (a b) -> q a b", a=HO),
                        in0=in0, scalar1=S_sb[:, p : p + 1],
                        scalar2=None, op0=Alu.mult,
                    )
                if k % 2 == 1:
                    pending.append(pp)
                if len(pending) > 1:
                    q = pending.pop(0)
                    nc.vector.tensor_tensor(
                        accW[:, :], q[:, :], accW[:, :], Alu.max,
                    )

        for q in pending:
            nc.vector.tensor_tensor(
                accW[:, :], q[:, :], accW[:, :], Alu.max,
            )
        acc12 = accW[:, 0 : 2 * FD]
        nc.vector.tensor_tensor(
            acc12, accW[:, 2 * FD : 4 * FD], acc12, Alu.max,
        )

        # Combine + transpose in one PE op per 128-pos chunk:
        #   pt = acc1_chunk.T @ [I;-I]  +  acc2_chunk.T @ [-I;I]
        #      = (accA1-accB1) - (accA2-accB2), position-major [cw, 64].
        # Then add the partition-replicated bias and DMA the chunk out.
        for c0 in range(0, FD, 128):
            cw = min(128, FD - c0)
            pt = tps.tile([128, COUT], F32)
            nc.tensor.matmul(pt[:cw, :], lhsT=accW[:, c0 : c0 + cw], rhs=M1_sb[:, :],
                             start=True, stop=False)
            nc.tensor.matmul(pt[:cw, :], lhsT=accW[:, FD + c0 : FD + c0 + cw], rhs=M2_sb[:, :],
                             start=False, stop=True)
            ysb = tsb.tile([128, COUT], F32)
            nc.vector.tensor_tensor(ysb[:cw, :], pt[:cw, :], BC_sb[:cw, :], Alu.add)
            nc.sync.dma_start(out=Y[c0 : c0 + cw, :], in_=ysb[:cw, :])
    nc.finalize()
    return nc


def _host_prep(x, k1, k2, bias):
    x = np.ascontiguousarray(np.asarray(x, dtype=np.float32))
    K1 = np.exp(np.asarray(k1, np.float32).reshape(P, COUT))
    K2 = np.exp(np.asarray(k2, np.float32).reshape(P, COUT))
    S1 = np.vstack([K1.T, K1.T]).astype(np.float32)          # [128, 288]
    S2 = np.vstack([K2.T, K2.T]).astype(np.float32)
    U1 = 0.1 * K1.max(axis=0)
    U2 = 0.1 * K2.max(axis=0)
    UB = np.stack([np.concatenate([U1, U1]), np.concatenate([U2, U2])], axis=1)
    UB = np.ascontiguousarray(UB, np.float32)                # [128, 2]
    BC = np.tile(np.asarray(bias, np.float32).reshape(1, COUT), (128, 1))
    PM = np.concatenate([np.ones(64, np.float32), -np.ones(64, np.float32)]).reshape(1, 128)
    M1 = np.vstack([np.eye(COUT, dtype=np.float16), -np.eye(COUT, dtype=np.float16)])
    M2 = np.ascontiguousarray(-M1)
    shared = dict(S1=S1, S2=S2, UB=UB, BC=np.ascontiguousarray(BC),
                  PM=np.ascontiguousarray(PM), M1=np.ascontiguousarray(M1), M2=M2)
    in_maps = []
    for n in range(N_CORES):
        xT = np.zeros((C, XLEN), np.float32)
        xT[:, :NPIX] = x[n].reshape(NPIX, C).T
        in_maps.append({"xT": xT.reshape(1, C * XLEN), **shared})
    return in_maps


def kernel(x, k1, k2, bias):
    global last_results
    if "nc" not in _cache:
        _cache["nc"] = _build_module()
    nc = _cache["nc"]
    in_maps = _host_prep(x, k1, k2, bias)
    trace = bool(int(os.environ.get("KTRACE", "0")))
    if trace:
        _ensure_axon_ntff_hook()
    res = run_bass_kernel_spmd(
        nc, in_maps, core_ids=list(range(N_CORES)), trace=trace,
    )
    last_results = res
    y = np.stack([r["Y"].reshape(HO, WO, COUT) for r in res.results], axis=0)
    return y.astype(np.float32)



# revision 3
# speedup vs baseline: 1.8739x; 1.8739x over previous
"""Bipolar morphological conv2d kernel for Trainium2 (8 NeuronCores).

Math: reference computes, per output position and out-channel c,
    y = m(lp1,K1) - m(lp1,K2) - m(lp2,K1) + m(lp2,K2) + bias
with m(logp, k)[c] = exp(max_p(logp_p + k_pc)), lp1 = log(max(patch, .1)),
lp2 = log(max(-patch, .1)).

Since exp is monotone, m(lp1,K)[c] = max(U_c, max_p(x_p*K_pc)) and
m(lp2,K)[c] = max(U_c, -min_p(x_p*K_pc)) with K = exp(k) > 0 and
U_c = .1*max_p K_pc (the clamp folds into a per-channel constant).  So the
whole op needs ONE product set per kernel, max- AND min-reduced over taps:
    y = (mA1 - mA2) + (aMin1 - aMin2) + bias
with mA_k = max(U_k, max_p x_p*K_k), aMin_k = min(-U_k, min_p x_p*K_k).
(The previous version materialized +x and -x product sets to use max-only
folds; computing each product once and folding max+min is strictly less
DVE work.)

Device strategy (data-parallel, one batch image per core):
  - partitions = 128 = [64 out-channels of K1 | 64 out-channels of K2]
  - free dim = 900 output positions as [30 rows, 30 cols] windows (row
    stride 32) into a per-ci broadcast row; host pre-replicates the rows
    across partitions in DRAM (fp16, even+odd parity copies so every tap
    window is 4B aligned). DMA streams them in; no PE broadcast needed.
  - products on the Activation engine (Copy with per-partition scale;
    Pool rejects TensorScalarPtr/TensorTensor at codegen, so only Act can
    offload them), grouped 4 taps per buffer; group 0 written straight
    into the accumulators.
  - folds on DVE: per group one tensor_tensor max + one min over
    [128, 4*900] fp16 (2x_1p mode). U clamp applied once post-merge.
  - combine: per 128-position chunk, two accumulating PE matmuls against
    [I;-I] compute (col_K1 - col_K2) of accMax plus same of accMin,
    transposed to position-major; add bias; DMA out.
"""

import os
from contextlib import ExitStack

import numpy as np

import concourse.bass as bass
import concourse.mybir as mybir
from concourse import bacc
import concourse.tile as tile
from concourse.bass_utils import run_bass_kernel_spmd

N_CORES = 8
H = W = C = 32
COUT = 64
HO = WO = 30
NPIX = H * W          # 1024
FD = HO * WO          # 900 output positions, accessed as [30, 30] windows
ROWL = 1026           # even-parity row length (1024 pixels + 2 pad)
XLEN = 2 * ROWL       # [even copy | odd (shifted-by-1) copy]
P = 288               # 3*3*32 patch size
G = 4                 # taps per product buffer / fold group

F32 = mybir.dt.float32
F16 = mybir.dt.float16
_cache: dict = {}
last_results = None


def _ensure_axon_ntff_hook():
    """The trimmed agent image lacks antenv.axon_hooks; recreate it so
    run_bass_kernel_spmd(trace=True) can capture NTFF profiles. No-op on
    failure (tracing then just degrades)."""
    import sys
    import types

    try:
        import antenv.axon_hooks  # noqa: F401
        return
    except ImportError:
        pass
    try:
        mod = types.ModuleType("antenv.axon_hooks")
        holder = [None]
        mod.set_axon_ntff_profile_hook = lambda h: holder.__setitem__(0, h)
        mod.get_axon_ntff_profile_hook = lambda: holder[0]
        sys.modules["antenv.axon_hooks"] = mod
        from trn_agent_boot.trn_boot import _ntff_profile_via_ctypes

        so = "/opt/axon/libaxon_pjrt.so"
        if os.path.exists(so):
            holder[0] = _ntff_profile_via_ctypes(so)
    except Exception:
        pass


def _build_module():
    nc = bacc.Bacc()
    Alu = mybir.AluOpType

    XB = nc.dram_tensor("XB", [C * 128, XLEN], F16, kind="ExternalInput")
    S = nc.dram_tensor("S", [128, P], F32, kind="ExternalInput")
    U2 = nc.dram_tensor("U2", [128, 2], F32, kind="ExternalInput")
    BC = nc.dram_tensor("BC", [128, COUT], F32, kind="ExternalInput")
    M1 = nc.dram_tensor("M1", [128, COUT], F16, kind="ExternalInput")
    Y = nc.dram_tensor("Y", [FD, COUT], F32, kind="ExternalOutput")

    with tile.TileContext(nc) as tc, ExitStack() as ctx:
        const = ctx.enter_context(tc.tile_pool(name="const", bufs=1))
        xbp = ctx.enter_context(tc.tile_pool(name="xbp", bufs=5))
        pbp = ctx.enter_context(tc.tile_pool(name="pbp", bufs=4))
        accp = ctx.enter_context(tc.tile_pool(name="accp", bufs=1))
        tps = ctx.enter_context(tc.tile_pool(name="tps", bufs=2, space="PSUM"))
        tsb = ctx.enter_context(tc.tile_pool(name="tsb", bufs=2))

        S_sb = const.tile([128, P], F32)
        nc.sync.dma_start(out=S_sb[:, :], in_=S[:, :])
        U2_sb = const.tile([128, 2], F32)
        nc.sync.dma_start(out=U2_sb[:, :], in_=U2[:, :])
        BC_sb = const.tile([128, COUT], F32)
        nc.sync.dma_start(out=BC_sb[:, :], in_=BC[:, :])
        M1_sb = const.tile([128, COUT], F16)
        nc.sync.dma_start(out=M1_sb[:, :], in_=M1[:, :])

        accMax = accp.tile([128, G * FD], F16)
        accMin = accp.tile([128, G * FD], F16)

        pb = None
        for ci in range(C):
            xb_sb = xbp.tile([128, XLEN], F16, tag="xb")
            nc.sync.dma_start(out=xb_sb[:, :], in_=XB[ci * 128 : (ci + 1) * 128, :])
            for t in range(9):
                i, j = divmod(t, 3)
                base = (ROWL + i * W) if j == 1 else (i * W + j)
                win = xb_sb[:, base : base + HO * W].rearrange(
                    "q (a b) -> q a b", b=W)[:, :, :WO]
                k = ci * 9 + t
                sc = S_sb[:, k : k + 1]
                g, slot = divmod(k, G)
                if g == 0:
                    # first group: products land directly in both accs
                    for acc in (accMax, accMin):
                        nc.scalar.mul(
                            out=acc[:, slot * FD : (slot + 1) * FD].rearrange(
                                "q (a b) -> q a b", a=HO),
                            in_=win, mul=sc)
                    continue
                if slot == 0:
                    pb = pbp.tile([128, G * FD], F16)
                nc.scalar.mul(
                    out=pb[:, slot * FD : (slot + 1) * FD].rearrange(
                        "q (a b) -> q a b", a=HO),
                    in_=win, mul=sc)
                if slot == G - 1:
                    nc.vector.tensor_tensor(
                        accMax[:, :], pb[:, :], accMax[:, :], Alu.max)
                    nc.vector.tensor_tensor(
                        accMin[:, :], pb[:, :], accMin[:, :], Alu.min)

        # merge sub-accumulators G -> G/2 -> 1, then clamp at +-U
        tmpx = accp.tile([128, 2 * FD], F16)
        tmpn = accp.tile([128, 2 * FD], F16)
        nc.vector.tensor_tensor(
            tmpx[:, :], accMax[:, : 2 * FD], accMax[:, 2 * FD :], Alu.max)
        nc.vector.tensor_tensor(
            tmpn[:, :], accMin[:, : 2 * FD], accMin[:, 2 * FD :], Alu.min)
        Mx = accp.tile([128, FD], F16)
        Mn = accp.tile([128, FD], F16)
        nc.vector.tensor_tensor(Mx[:, :], tmpx[:, :FD], tmpx[:, FD:], Alu.max)
        nc.vector.tensor_tensor(Mn[:, :], tmpn[:, :FD], tmpn[:, FD:], Alu.min)
        nc.vector.tensor_scalar(
            out=Mx[:, :], in0=Mx[:, :],
            scalar1=U2_sb[:, 0:1], scalar2=None, op0=Alu.max)
        nc.vector.tensor_scalar(
            out=Mn[:, :], in0=Mn[:, :],
            scalar1=U2_sb[:, 1:2], scalar2=None, op0=Alu.min)

        # Combine + transpose per 128-pos chunk:
        #   pt = Mx_chunk.T @ [I;-I] + Mn_chunk.T @ [I;-I]
        #      = (mA1-mA2) + (aMin1-aMin2), position-major [cw, 64].
        for c0 in range(0, FD, 128):
            cw = min(128, FD - c0)
            pt = tps.tile([128, COUT], F32)
            nc.tensor.matmul(pt[:cw, :], lhsT=Mx[:, c0 : c0 + cw], rhs=M1_sb[:, :],
                             start=True, stop=False)
            nc.tensor.matmul(pt[:cw, :], lhsT=Mn[:, c0 : c0 + cw], rhs=M1_sb[:, :],
                             start=False, stop=True)
            ysb = tsb.tile([128, COUT], F32)
            nc.vector.tensor_tensor(ysb[:cw, :], pt[:cw, :], BC_sb[:cw, :], Alu.add)
            nc.sync.dma_start(out=Y[c0 : c0 + cw, :], in_=ysb[:cw, :])
    nc.finalize()
    return nc


def _host_prep(x, k1, k2, bias):
    x = np.ascontiguousarray(np.asarray(x, dtype=np.float32))
    K1 = np.exp(np.asarray(k1, np.float32).reshape(3, 3, C, COUT))
    K2 = np.exp(np.asarray(k2, np.float32).reshape(3, 3, C, COUT))
    # S[q, ci*9 + i*3 + j]: q<64 -> K1[i,j,ci,q];  q>=64 -> K2[i,j,ci,q-64]
    S1 = K1.transpose(3, 2, 0, 1).reshape(COUT, P)
    S2 = K2.transpose(3, 2, 0, 1).reshape(COUT, P)
    S = np.ascontiguousarray(np.vstack([S1, S2]), np.float32)
    U1 = 0.1 * K1.reshape(9 * C, COUT).max(axis=0)
    U2_ = 0.1 * K2.reshape(9 * C, COUT).max(axis=0)
    U = np.concatenate([U1, U2_])
    U2 = np.ascontiguousarray(np.stack([U, -U], axis=1), np.float32)  # [128,2]
    BC = np.tile(np.asarray(bias, np.float32).reshape(1, COUT), (128, 1))
    M1 = np.vstack([np.eye(COUT, dtype=np.float16), -np.eye(COUT, dtype=np.float16)])
    shared = dict(S=S, U2=U2, BC=np.ascontiguousarray(BC),
                  M1=np.ascontiguousarray(M1))
    in_maps = []
    for n in range(N_CORES):
        rows = np.zeros((C, XLEN), np.float16)
        xr = x[n].reshape(NPIX, C).T.astype(np.float16)       # [C, 1024]
        rows[:, :NPIX] = xr
        rows[:, ROWL : ROWL + NPIX - 1] = xr[:, 1:]
        xb = np.broadcast_to(rows[:, None, :], (C, 128, XLEN))
        in_maps.append({"XB": np.ascontiguousarray(xb).reshape(C * 128, XLEN),
                        **shared})
    return in_maps


def kernel(x, k1, k2, bias):
    global last_results
    if "nc" not in _cache:
        _cache["nc"] = _build_module()
    nc = _cache["nc"]
    in_maps = _host_prep(x, k1, k2, bias)
    trace = bool(int(os.environ.get("KTRACE", "0")))
    if trace:
        _ensure_axon_ntff_hook()
    res = run_bass_kernel_spmd(
        nc, in_maps, core_ids=list(range(N_CORES)), trace=trace,
    )
    last_results = res
    y = np.stack([r["Y"].reshape(HO, WO, COUT) for r in res.results], axis=0)
    return y.astype(np.float32)


# revision 5
# speedup vs baseline: 1.9370x; 1.0337x over previous
"""Bipolar morphological conv2d kernel for Trainium2 (8 NeuronCores).

Math: reference computes, per output position and out-channel c,
    y = m(lp1,K1) - m(lp1,K2) - m(lp2,K1) + m(lp2,K2) + bias
with m(logp, k)[c] = exp(max_p(logp_p + k_pc)), lp1 = log(max(patch, .1)),
lp2 = log(max(-patch, .1)).

Since exp is monotone, m(lp1,K)[c] = max(U_c, max_p(x_p*K_pc)) and
m(lp2,K)[c] = max(U_c, -min_p(x_p*K_pc)) with K = exp(k) > 0 and
U_c = .1*max_p K_pc (the clamp folds into a per-channel constant).  So the
whole op needs ONE product set per kernel, max- AND min-reduced over taps:
    y = (mA1 - mA2) + (aMin1 - aMin2) + bias
with mA_k = max(U_k, max_p x_p*K_k), aMin_k = min(-U_k, min_p x_p*K_k).
(The previous version materialized +x and -x product sets to use max-only
folds; computing each product once and folding max+min is strictly less
DVE work.)

Device strategy (data-parallel, one batch image per core):
  - partitions = 128 = [64 out-channels of K1 | 64 out-channels of K2]
  - free dim = 900 output positions as [30 rows, 30 cols] windows (row
    stride 32) into a per-ci broadcast row; host pre-replicates the rows
    across partitions in DRAM (fp16, even+odd parity copies so every tap
    window is 4B aligned). DMA streams them in; no PE broadcast needed.
  - products on the Activation engine (Copy with per-partition scale;
    Pool rejects TensorScalarPtr/TensorTensor at codegen, so only Act can
    offload them), grouped 4 taps per buffer; group 0 written straight
    into the accumulators.
  - folds on DVE: per group one tensor_tensor max + one min over
    [128, 4*900] fp16 (2x_1p mode). U clamp applied once post-merge.
  - combine: per 128-position chunk, two accumulating PE matmuls against
    [I;-I] compute (col_K1 - col_K2) of accMax plus same of accMin,
    transposed to position-major; add bias; DMA out.
"""

import os
from contextlib import ExitStack

import numpy as np

import concourse.bass as bass
import concourse.mybir as mybir
from concourse import bacc
import concourse.tile as tile
from concourse.bass_utils import run_bass_kernel_spmd

N_CORES = 8
H = W = C = 32
COUT = 64
HO = WO = 30
NPIX = H * W          # 1024
FD = HO * WO          # 900 output positions, accessed as [30, 30] windows
ROWL = 1026           # even-parity row length (1024 pixels + 2 pad)
XLEN = 2 * ROWL       # [even copy | odd (shifted-by-1) copy]
P = 288               # 3*3*32 patch size
G = 4                 # taps per product buffer / fold group

F32 = mybir.dt.float32
F16 = mybir.dt.float16
_cache: dict = {}
last_results = None


def _ensure_axon_ntff_hook():
    """The trimmed agent image lacks antenv.axon_hooks; recreate it so
    run_bass_kernel_spmd(trace=True) can capture NTFF profiles. No-op on
    failure (tracing then just degrades)."""
    import sys
    import types

    try:
        import antenv.axon_hooks  # noqa: F401
        return
    except ImportError:
        pass
    try:
        mod = types.ModuleType("antenv.axon_hooks")
        holder = [None]
        mod.set_axon_ntff_profile_hook = lambda h: holder.__setitem__(0, h)
        mod.get_axon_ntff_profile_hook = lambda: holder[0]
        sys.modules["antenv.axon_hooks"] = mod
        from trn_agent_boot.trn_boot import _ntff_profile_via_ctypes

        so = "/opt/axon/libaxon_pjrt.so"
        if os.path.exists(so):
            holder[0] = _ntff_profile_via_ctypes(so)
    except Exception:
        pass


def _build_module():
    nc = bacc.Bacc()
    Alu = mybir.AluOpType

    XB = nc.dram_tensor("XB", [C * 128, XLEN], F16, kind="ExternalInput")
    S = nc.dram_tensor("S", [128, P], F32, kind="ExternalInput")
    U2 = nc.dram_tensor("U2", [128, 2], F32, kind="ExternalInput")
    BC = nc.dram_tensor("BC", [128, COUT], F32, kind="ExternalInput")
    M1 = nc.dram_tensor("M1", [128, COUT], F16, kind="ExternalInput")
    Y = nc.dram_tensor("Y", [FD, COUT], F32, kind="ExternalOutput")

    with tile.TileContext(nc) as tc, ExitStack() as ctx:
        const = ctx.enter_context(tc.tile_pool(name="const", bufs=1))
        xbp = ctx.enter_context(tc.tile_pool(name="xbp", bufs=5))
        pbp = ctx.enter_context(tc.tile_pool(name="pbp", bufs=4))
        accp = ctx.enter_context(tc.tile_pool(name="accp", bufs=1))
        tps = ctx.enter_context(tc.tile_pool(name="tps", bufs=2, space="PSUM"))
        tsb = ctx.enter_context(tc.tile_pool(name="tsb", bufs=2))

        S_sb = const.tile([128, P], F32)
        nc.sync.dma_start(out=S_sb[:, :], in_=S[:, :])
        U2_sb = const.tile([128, 2], F32)
        nc.sync.dma_start(out=U2_sb[:, :], in_=U2[:, :])
        BC_sb = const.tile([128, COUT], F32)
        nc.sync.dma_start(out=BC_sb[:, :], in_=BC[:, :])
        M1_sb = const.tile([128, COUT], F16)
        nc.sync.dma_start(out=M1_sb[:, :], in_=M1[:, :])

        accMax = accp.tile([128, G * FD], F16)
        accMin = accp.tile([128, G * FD], F16)

        pb = None
        for ci in range(C):
            xb_sb = xbp.tile([128, XLEN], F16, tag="xb")
            nc.sync.dma_start(out=xb_sb[:, :], in_=XB[ci * 128 : (ci + 1) * 128, :])
            for t in range(9):
                i, j = divmod(t, 3)
                base = (ROWL + i * W) if j == 1 else (i * W + j)
                win = xb_sb[:, base : base + HO * W].rearrange(
                    "q (a b) -> q a b", b=W)[:, :, :WO]
                k = ci * 9 + t
                sc = S_sb[:, k : k + 1]
                g, slot = divmod(k, G)
                if g == 0:
                    # first group seeds accMax directly; accMin is copied
                    # from it once (below) instead of duplicating products
                    dst = accMax
                elif slot == 0:
                    pb = pbp.tile([128, G * FD], F16, tag="pb")
                    dst = pb
                else:
                    dst = pb
                out_view = dst[:, slot * FD : (slot + 1) * FD].rearrange(
                    "q (a b) -> q a b", a=HO)
                if k % 36 == 35:
                    # a sliver of products runs on DVE to balance Act
                    nc.vector.tensor_scalar(
                        out=out_view, in0=win, scalar1=sc, scalar2=None,
                        op0=Alu.mult)
                else:
                    nc.scalar.mul(out=out_view, in_=win, mul=sc)
                if g == 0 and slot == G - 1:
                    nc.vector.tensor_scalar(
                        out=accMin[:, :], in0=accMax[:, :], scalar1=0.0,
                        scalar2=None, op0=Alu.add)
                if g > 0 and slot == G - 1:
                    nc.vector.tensor_tensor(
                        accMax[:, :], pb[:, :], accMax[:, :], Alu.max)
                    nc.vector.tensor_tensor(
                        accMin[:, :], pb[:, :], accMin[:, :], Alu.min)

        # merge sub-accumulators G -> G/2 -> 1, then clamp at +-U
        tmpx = accp.tile([128, 2 * FD], F16)
        tmpn = accp.tile([128, 2 * FD], F16)
        nc.vector.tensor_tensor(
            tmpx[:, :], accMax[:, : 2 * FD], accMax[:, 2 * FD :], Alu.max)
        nc.vector.tensor_tensor(
            tmpn[:, :], accMin[:, : 2 * FD], accMin[:, 2 * FD :], Alu.min)
        Mx = accp.tile([128, FD], F16)
        Mn = accp.tile([128, FD], F16)
        nc.vector.tensor_tensor(Mx[:, :], tmpx[:, :FD], tmpx[:, FD:], Alu.max)
        nc.vector.tensor_tensor(Mn[:, :], tmpn[:, :FD], tmpn[:, FD:], Alu.min)
        nc.vector.tensor_scalar(
            out=Mx[:, :], in0=Mx[:, :],
            scalar1=U2_sb[:, 0:1], scalar2=None, op0=Alu.max)
        nc.vector.tensor_scalar(
            out=Mn[:, :], in0=Mn[:, :],
            scalar1=U2_sb[:, 1:2], scalar2=None, op0=Alu.min)

        # Combine + transpose per 128-pos chunk:
        #   pt = Mx_chunk.T @ [I;-I] + Mn_chunk.T @ [I;-I]
        #      = (mA1-mA2) + (aMin1-aMin2), position-major [cw, 64].
        for c0 in range(0, FD, 128):
            cw = min(128, FD - c0)
            pt = tps.tile([128, COUT], F32)
            nc.tensor.matmul(pt[:cw, :], lhsT=Mx[:, c0 : c0 + cw], rhs=M1_sb[:, :],
                             start=True, stop=False)
            nc.tensor.matmul(pt[:cw, :], lhsT=Mn[:, c0 : c0 + cw], rhs=M1_sb[:, :],
                             start=False, stop=True)
            ysb = tsb.tile([128, COUT], F32)
            nc.vector.tensor_tensor(ysb[:cw, :], pt[:cw, :], BC_sb[:cw, :], Alu.add)
            nc.sync.dma_start(out=Y[c0 : c0 + cw, :], in_=ysb[:cw, :])
    nc.finalize()
    return nc


def _host_prep(x, k1, k2, bias):
    x = np.ascontiguousarray(np.asarray(x, dtype=np.float32))
    K1 = np.exp(np.asarray(k1, np.float32).reshape(3, 3, C, COUT))
    K2 = np.exp(np.asarray(k2, np.float32).reshape(3, 3, C, COUT))
    # S[q, ci*9 + i*3 + j]: q<64 -> K1[i,j,ci,q];  q>=64 -> K2[i,j,ci,q-64]
    S1 = K1.transpose(3, 2, 0, 1).reshape(COUT, P)
    S2 = K2.transpose(3, 2, 0, 1).reshape(COUT, P)
    S = np.ascontiguousarray(np.vstack([S1, S2]), np.float32)
    U1 = 0.1 * K1.reshape(9 * C, COUT).max(axis=0)
    U2_ = 0.1 * K2.reshape(9 * C, COUT).max(axis=0)
    U = np.concatenate([U1, U2_])
    U2 = np.ascontiguousarray(np.stack([U, -U], axis=1), np.float32)  # [128,2]
    BC = np.tile(np.asarray(bias, np.float32).reshape(1, COUT), (128, 1))
    M1 = np.vstack([np.eye(COUT, dtype=np.float16), -np.eye(COUT, dtype=np.float16)])
    shared = dict(S=S, U2=U2, BC=np.ascontiguousarray(BC),
                  M1=np.ascontiguousarray(M1))
    in_maps = []
    for n in range(N_CORES):
        rows = np.zeros((C, XLEN), np.float16)
        xr = x[n].reshape(NPIX, C).T.astype(np.float16)       # [C, 1024]
        rows[:, :NPIX] = xr
        rows[:, ROWL : ROWL + NPIX - 1] = xr[:, 1:]
        xb = np.broadcast_to(rows[:, None, :], (C, 128, XLEN))
        in_maps.append({"XB": np.ascontiguousarray(xb).reshape(C * 128, XLEN),
                        **shared})
    return in_maps


def kernel(x, k1, k2, bias):
    global last_results
    if "nc" not in _cache:
        _cache["nc"] = _build_module()
    nc = _cache["nc"]
    in_maps = _host_prep(x, k1, k2, bias)
    trace = bool(int(os.environ.get("KTRACE", "0")))
    if trace:
        _ensure_axon_ntff_hook()
    res = run_bass_kernel_spmd(
        nc, in_maps, core_ids=list(range(N_CORES)), trace=trace,
    )
    last_results = res
    y = np.stack([r["Y"].reshape(HO, WO, COUT) for r in res.results], axis=0)
    return y.astype(np.float32)


# revision 8
# speedup vs baseline: 1.9476x; 1.0054x over previous
"""Bipolar morphological conv2d kernel for Trainium2 (8 NeuronCores).

Math: reference computes, per output position and out-channel c,
    y = m(lp1,K1) - m(lp1,K2) - m(lp2,K1) + m(lp2,K2) + bias
with m(logp, k)[c] = exp(max_p(logp_p + k_pc)), lp1 = log(max(patch, .1)),
lp2 = log(max(-patch, .1)).

Since exp is monotone, m(lp1,K)[c] = max(U_c, max_p(x_p*K_pc)) and
m(lp2,K)[c] = max(U_c, -min_p(x_p*K_pc)) with K = exp(k) > 0 and
U_c = .1*max_p K_pc (the clamp folds into a per-channel constant).  So the
whole op needs ONE product set per kernel, max- AND min-reduced over taps:
    y = (mA1 - mA2) + (aMin1 - aMin2) + bias
with mA_k = max(U_k, max_p x_p*K_k), aMin_k = min(-U_k, min_p x_p*K_k).
(The previous version materialized +x and -x product sets to use max-only
folds; computing each product once and folding max+min is strictly less
DVE work.)

Device strategy (data-parallel, one batch image per core):
  - partitions = 128 = [64 out-channels of K1 | 64 out-channels of K2]
  - free dim = 900 output positions as [30 rows, 30 cols] windows (row
    stride 32) into a per-ci broadcast row; host pre-replicates the rows
    across partitions in DRAM (fp16, even+odd parity copies so every tap
    window is 4B aligned). DMA streams them in; no PE broadcast needed.
  - products on the Activation engine (Copy with per-partition scale;
    Pool rejects TensorScalarPtr/TensorTensor at codegen, so only Act can
    offload them), grouped 4 taps per buffer; group 0 written straight
    into the accumulators.
  - folds on DVE: per group one tensor_tensor max + one min over
    [128, 4*900] fp16 (2x_1p mode). U clamp applied once post-merge.
  - combine: per 128-position chunk, two accumulating PE matmuls against
    [I;-I] compute (col_K1 - col_K2) of accMax plus same of accMin,
    transposed to position-major; add bias; DMA out.
"""

import os
from contextlib import ExitStack

import numpy as np

import concourse.bass as bass
import concourse.mybir as mybir
from concourse import bacc
import concourse.tile as tile
from concourse.bass_utils import run_bass_kernel_spmd

N_CORES = 8
H = W = C = 32
COUT = 64
HO = WO = 30
NPIX = H * W          # 1024
FD = HO * WO          # 900 output positions, accessed as [30, 30] windows
ROWL = 1026           # even-parity row length (1024 pixels + 2 pad)
XLEN = 2 * ROWL       # [even copy | odd (shifted-by-1) copy]
P = 288               # 3*3*32 patch size
G = 4                 # taps per product buffer / fold group

F32 = mybir.dt.float32
F16 = mybir.dt.float16
_cache: dict = {}
last_results = None


def _ensure_axon_ntff_hook():
    """The trimmed agent image lacks antenv.axon_hooks; recreate it so
    run_bass_kernel_spmd(trace=True) can capture NTFF profiles. No-op on
    failure (tracing then just degrades)."""
    import sys
    import types

    try:
        import antenv.axon_hooks  # noqa: F401
        return
    except ImportError:
        pass
    try:
        mod = types.ModuleType("antenv.axon_hooks")
        holder = [None]
        mod.set_axon_ntff_profile_hook = lambda h: holder.__setitem__(0, h)
        mod.get_axon_ntff_profile_hook = lambda: holder[0]
        sys.modules["antenv.axon_hooks"] = mod
        from trn_agent_boot.trn_boot import _ntff_profile_via_ctypes

        so = "/opt/axon/libaxon_pjrt.so"
        if os.path.exists(so):
            holder[0] = _ntff_profile_via_ctypes(so)
    except Exception:
        pass


def _build_module():
    nc = bacc.Bacc()
    Alu = mybir.AluOpType

    XB = nc.dram_tensor("XB", [C * 128, XLEN], F16, kind="ExternalInput")
    S = nc.dram_tensor("S", [128, P], F32, kind="ExternalInput")
    U2 = nc.dram_tensor("U2", [128, 2], F32, kind="ExternalInput")
    BC = nc.dram_tensor("BC", [128, COUT], F32, kind="ExternalInput")
    M1 = nc.dram_tensor("M1", [128, COUT], F16, kind="ExternalInput")
    Y = nc.dram_tensor("Y", [FD, COUT], F32, kind="ExternalOutput")

    with tile.TileContext(nc) as tc, ExitStack() as ctx:
        const = ctx.enter_context(tc.tile_pool(name="const", bufs=1))
        xbp = ctx.enter_context(tc.tile_pool(name="xbp", bufs=5))
        pbp = ctx.enter_context(tc.tile_pool(name="pbp", bufs=6))
        accp = ctx.enter_context(tc.tile_pool(name="accp", bufs=1))
        tps = ctx.enter_context(tc.tile_pool(name="tps", bufs=2, space="PSUM"))
        tsb = ctx.enter_context(tc.tile_pool(name="tsb", bufs=2))

        # xb[0] + S gate the first products: issue them first, on the sync
        # queue; the other consts (needed only much later) go via gpsimd.
        xb0 = xbp.tile([128, XLEN], F16, tag="xb")
        nc.sync.dma_start(out=xb0[:, :], in_=XB[0:128, :])
        S_sb = const.tile([128, P], F32)
        nc.sync.dma_start(out=S_sb[:, :], in_=S[:, :])
        U2_sb = const.tile([128, 2], F32)
        nc.gpsimd.dma_start(out=U2_sb[:, :], in_=U2[:, :])
        BC_sb = const.tile([128, COUT], F32)
        nc.gpsimd.dma_start(out=BC_sb[:, :], in_=BC[:, :])
        M1_sb = const.tile([128, COUT], F16)
        nc.gpsimd.dma_start(out=M1_sb[:, :], in_=M1[:, :])

        accMax = accp.tile([128, G * FD], F16)
        accMin = accp.tile([128, G * FD], F16)

        pb = None
        for ci in range(C):
            if ci == 0:
                xb_sb = xb0
            else:
                xb_sb = xbp.tile([128, XLEN], F16, tag="xb")
                nc.sync.dma_start(
                    out=xb_sb[:, :], in_=XB[ci * 128 : (ci + 1) * 128, :])
            for t in range(9):
                i, j = divmod(t, 3)
                base = (ROWL + i * W) if j == 1 else (i * W + j)
                win = xb_sb[:, base : base + HO * W].rearrange(
                    "q (a b) -> q a b", b=W)[:, :, :WO]
                k = ci * 9 + t
                sc = S_sb[:, k : k + 1]
                g, slot = divmod(k, G)
                if g == 0:
                    # first group seeds accMax directly; accMin is copied
                    # from it once (below) instead of duplicating products
                    dst = accMax
                elif slot == 0:
                    pb = pbp.tile([128, G * FD], F16, tag="pb")
                    dst = pb
                else:
                    dst = pb
                out_view = dst[:, slot * FD : (slot + 1) * FD].rearrange(
                    "q (a b) -> q a b", a=HO)
                if g == 0 or k % 72 == 71:
                    # seed group + a sliver of products run on DVE: it is
                    # idle during ramp-up and slightly under Act's load
                    nc.vector.tensor_scalar(
                        out=out_view, in0=win, scalar1=sc, scalar2=None,
                        op0=Alu.mult)
                else:
                    nc.scalar.mul(out=out_view, in_=win, mul=sc)
                if g == 0 and slot == G - 1:
                    nc.vector.tensor_scalar(
                        out=accMin[:, :], in0=accMax[:, :], scalar1=0.0,
                        scalar2=None, op0=Alu.add)
                if g > 0 and slot == G - 1:
                    nc.vector.tensor_tensor(
                        accMax[:, :], pb[:, :], accMax[:, :], Alu.max)
                    nc.vector.tensor_tensor(
                        accMin[:, :], pb[:, :], accMin[:, :], Alu.min)

        # merge sub-accumulators G -> G/2 -> 1, then clamp at +-U
        tmpx = accp.tile([128, 2 * FD], F16)
        tmpn = accp.tile([128, 2 * FD], F16)
        nc.vector.tensor_tensor(
            tmpx[:, :], accMax[:, : 2 * FD], accMax[:, 2 * FD :], Alu.max)
        nc.vector.tensor_tensor(
            tmpn[:, :], accMin[:, : 2 * FD], accMin[:, 2 * FD :], Alu.min)
        Mx = accp.tile([128, FD], F16)
        Mn = accp.tile([128, FD], F16)
        nc.vector.tensor_tensor(Mx[:, :], tmpx[:, :FD], tmpx[:, FD:], Alu.max)
        nc.vector.tensor_tensor(Mn[:, :], tmpn[:, :FD], tmpn[:, FD:], Alu.min)
        nc.vector.tensor_scalar(
            out=Mx[:, :], in0=Mx[:, :],
            scalar1=U2_sb[:, 0:1], scalar2=None, op0=Alu.max)
        nc.vector.tensor_scalar(
            out=Mn[:, :], in0=Mn[:, :],
            scalar1=U2_sb[:, 1:2], scalar2=None, op0=Alu.min)

        # Combine + transpose per 128-pos chunk:
        #   pt = Mx_chunk.T @ [I;-I] + Mn_chunk.T @ [I;-I]
        #      = (mA1-mA2) + (aMin1-aMin2), position-major [cw, 64].
        for c0 in range(0, FD, 128):
            cw = min(128, FD - c0)
            pt = tps.tile([128, COUT], F32)
            nc.tensor.matmul(pt[:cw, :], lhsT=Mx[:, c0 : c0 + cw], rhs=M1_sb[:, :],
                             start=True, stop=False)
            nc.tensor.matmul(pt[:cw, :], lhsT=Mn[:, c0 : c0 + cw], rhs=M1_sb[:, :],
                             start=False, stop=True)
            ysb = tsb.tile([128, COUT], F32)
            nc.vector.tensor_tensor(ysb[:cw, :], pt[:cw, :], BC_sb[:cw, :], Alu.add)
            nc.sync.dma_start(out=Y[c0 : c0 + cw, :], in_=ysb[:cw, :])
    nc.finalize()
    return nc


def _host_prep(x, k1, k2, bias):
    x = np.ascontiguousarray(np.asarray(x, dtype=np.float32))
    K1 = np.exp(np.asarray(k1, np.float32).reshape(3, 3, C, COUT))
    K2 = np.exp(np.asarray(k2, np.float32).reshape(3, 3, C, COUT))
    # S[q, ci*9 + i*3 + j]: q<64 -> K1[i,j,ci,q];  q>=64 -> K2[i,j,ci,q-64]
    S1 = K1.transpose(3, 2, 0, 1).reshape(COUT, P)
    S2 = K2.transpose(3, 2, 0, 1).reshape(COUT, P)
    S = np.ascontiguousarray(np.vstack([S1, S2]), np.float32)
    U1 = 0.1 * K1.reshape(9 * C, COUT).max(axis=0)
    U2_ = 0.1 * K2.reshape(9 * C, COUT).max(axis=0)
    U = np.concatenate([U1, U2_])
    U2 = np.ascontiguousarray(np.stack([U, -U], axis=1), np.float32)  # [128,2]
    BC = np.tile(np.asarray(bias, np.float32).reshape(1, COUT), (128, 1))
    M1 = np.vstack([np.eye(COUT, dtype=np.float16), -np.eye(COUT, dtype=np.float16)])
    shared = dict(S=S, U2=U2, BC=np.ascontiguousarray(BC),
                  M1=np.ascontiguousarray(M1))
    in_maps = []
    for n in range(N_CORES):
        rows = np.zeros((C, XLEN), np.float16)
        xr = x[n].reshape(NPIX, C).T.astype(np.float16)       # [C, 1024]
        rows[:, :NPIX] = xr
        rows[:, ROWL : ROWL + NPIX - 1] = xr[:, 1:]
        xb = np.broadcast_to(rows[:, None, :], (C, 128, XLEN))
        in_maps.append({"XB": np.ascontiguousarray(xb).reshape(C * 128, XLEN),
                        **shared})
    return in_maps


def kernel(x, k1, k2, bias):
    global last_results
    if "nc" not in _cache:
        _cache["nc"] = _build_module()
    nc = _cache["nc"]
    in_maps = _host_prep(x, k1, k2, bias)
    trace = bool(int(os.environ.get("KTRACE", "0")))
    if trace:
        _ensure_axon_ntff_hook()
    res = run_bass_kernel_spmd(
        nc, in_maps, core_ids=list(range(N_CORES)), trace=trace,
    )
    last_results = res
    y = np.stack([r["Y"].reshape(HO, WO, COUT) for r in res.results], axis=0)
    return y.astype(np.float32)


# revision 10
# speedup vs baseline: 1.9937x; 1.0237x over previous
"""Bipolar morphological conv2d kernel for Trainium2 (8 NeuronCores).

Math: reference computes, per output position and out-channel c,
    y = m(lp1,K1) - m(lp1,K2) - m(lp2,K1) + m(lp2,K2) + bias
with m(logp, k)[c] = exp(max_p(logp_p + k_pc)), lp1 = log(max(patch, .1)),
lp2 = log(max(-patch, .1)).

Since exp is monotone, m(lp1,K)[c] = max(U_c, max_p(x_p*K_pc)) and
m(lp2,K)[c] = max(U_c, -min_p(x_p*K_pc)) with K = exp(k) > 0 and
U_c = .1*max_p K_pc (the clamp folds into a per-channel constant).  So the
whole op needs ONE product set per kernel, max- AND min-reduced over taps:
    y = (mA1 - mA2) + (aMin1 - aMin2) + bias
with mA_k = max(U_k, max_p x_p*K_k), aMin_k = min(-U_k, min_p x_p*K_k).

Device strategy (data-parallel, one batch image per core):
  - partitions = 128 = [64 out-channels of K1 | 64 out-channels of K2]
  - free dim = 900 output positions as [30 rows, 30 cols] windows (row
    stride 32) into a per-ci broadcast row; host pre-replicates the rows
    across partitions in DRAM (fp16, even+odd parity copies so every tap
    window is 4B aligned).  The per-(tap,ci) kernel scalars and the U
    clamps ride along as trailing columns of the same tile, so one DMA
    per ci delivers everything and nothing else gates the first product.
  - products on the Activation engine (Copy with per-partition scale;
    Pool rejects TensorScalarPtr/TensorTensor at codegen), grouped 4 taps
    per buffer; the seed group and a small sliver run on DVE to balance.
  - folds on DVE: per group one tensor_tensor max + one min over
    [128, 4*900] fp16 (2x_1p mode). U clamp applied once post-merge.
  - combine: two accumulating PE matmuls lhsT=[I;-I] turn the merged accs
    into (col_K1 - col_K2) sums with channels on PARTITIONS ([64, 900]
    PSUM), Activation adds the bias while staging PSUM->SBUF, and one
    64-descriptor DMA writes Y [64, 900] (host transposes).
"""

import os
from contextlib import ExitStack

import numpy as np

import concourse.bass as bass
import concourse.mybir as mybir
from concourse import bacc
import concourse.tile as tile
from concourse.bass_utils import run_bass_kernel_spmd

N_CORES = 8
H = W = C = 32
COUT = 64
HO = WO = 30
NPIX = H * W          # 1024
FD = HO * WO          # 900 output positions, accessed as [30, 30] windows
ROWL = 1026           # even-parity row length (1024 pixels + 2 pad)
SCOL = 2 * ROWL       # fp32 per-tap scalars, packed as fp16 slot pairs
UCOL = SCOL + 18      # [U, -U] fp32 columns (2 slots each)
XLEN = UCOL + 4       # row length in fp16 slots (4B aligned)
P = 288               # 3*3*32 patch size
G = 4                 # taps per product buffer / fold group

F32 = mybir.dt.float32
F16 = mybir.dt.float16
_cache: dict = {}
last_results = None


def _ensure_axon_ntff_hook():
    """The trimmed agent image lacks antenv.axon_hooks; recreate it so
    run_bass_kernel_spmd(trace=True) can capture NTFF profiles. No-op on
    failure (tracing then just degrades)."""
    import sys
    import types

    try:
        import antenv.axon_hooks  # noqa: F401
        return
    except ImportError:
        pass
    try:
        mod = types.ModuleType("antenv.axon_hooks")
        holder = [None]
        mod.set_axon_ntff_profile_hook = lambda h: holder.__setitem__(0, h)
        mod.get_axon_ntff_profile_hook = lambda: holder[0]
        sys.modules["antenv.axon_hooks"] = mod
        from trn_agent_boot.trn_boot import _ntff_profile_via_ctypes

        so = "/opt/axon/libaxon_pjrt.so"
        if os.path.exists(so):
            holder[0] = _ntff_profile_via_ctypes(so)
    except Exception:
        pass


def _build_module():
    nc = bacc.Bacc()
    Alu = mybir.AluOpType

    XB = nc.dram_tensor("XB", [C * 128, XLEN], F16, kind="ExternalInput")
    M1 = nc.dram_tensor("M1", [128, COUT], F16, kind="ExternalInput")
    BCc = nc.dram_tensor("BCc", [COUT, 1], F32, kind="ExternalInput")
    Y = nc.dram_tensor("Y", [COUT, FD], F32, kind="ExternalOutput")

    with tile.TileContext(nc) as tc, ExitStack() as ctx:
        const = ctx.enter_context(tc.tile_pool(name="const", bufs=1))
        xbp = ctx.enter_context(tc.tile_pool(name="xbp", bufs=5))
        pbp = ctx.enter_context(tc.tile_pool(name="pbp", bufs=6))
        accp = ctx.enter_context(tc.tile_pool(name="accp", bufs=1))
        tps = ctx.enter_context(tc.tile_pool(name="tps", bufs=1, space="PSUM"))
        tsb = ctx.enter_context(tc.tile_pool(name="tsb", bufs=1))

        # xb[0] gates the first products: issue it before everything else.
        xb0 = xbp.tile([128, XLEN], F16, tag="xb")
        nc.sync.dma_start(out=xb0[:, :], in_=XB[0:128, :])
        M1_sb = const.tile([128, COUT], F16)
        nc.gpsimd.dma_start(out=M1_sb[:, :], in_=M1[:, :])
        BC_sb = const.tile([COUT, 1], F32)
        nc.gpsimd.dma_start(out=BC_sb[:, :], in_=BCc[:, :])

        accMax = accp.tile([128, G * FD], F16)
        accMin = accp.tile([128, G * FD], F16)

        pb = None
        xbf = xb0  # tile holding the (identical) U columns, kept live
        for ci in range(C):
            if ci == 0:
                xb_sb = xb0
            else:
                xb_sb = xbp.tile([128, XLEN], F16, tag="xb")
                nc.sync.dma_start(
                    out=xb_sb[:, :], in_=XB[ci * 128 : (ci + 1) * 128, :])
                if ci == C - 1:
                    xbf = xb_sb
            for t in range(9):
                i, j = divmod(t, 3)
                base = (ROWL + i * W) if j == 1 else (i * W + j)
                win = xb_sb[:, base : base + HO * W].rearrange(
                    "q (a b) -> q a b", b=W)[:, :, :WO]
                k = ci * 9 + t
                sc = xb_sb[:, SCOL + 2 * t : SCOL + 2 * t + 2].bitcast(F32)
                g, slot = divmod(k, G)
                if g == 0:
                    # first group seeds accMax directly; accMin is copied
                    # from it once (below) instead of duplicating products
                    dst = accMax
                elif slot == 0:
                    pb = pbp.tile([128, G * FD], F16, tag="pb")
                    dst = pb
                else:
                    dst = pb
                out_view = dst[:, slot * FD : (slot + 1) * FD].rearrange(
                    "q (a b) -> q a b", a=HO)
                if g == 0 or k % 72 == 71:
                    # seed group + a sliver of products run on DVE: it is
                    # idle during ramp-up and slightly under Act's load
                    nc.vector.tensor_scalar(
                        out=out_view, in0=win, scalar1=sc, scalar2=None,
                        op0=Alu.mult)
                else:
                    nc.scalar.mul(out=out_view, in_=win, mul=sc)
                if g == 0 and slot == G - 1:
                    nc.vector.tensor_scalar(
                        out=accMin[:, :], in0=accMax[:, :], scalar1=0.0,
                        scalar2=None, op0=Alu.add)
                if g > 0 and slot == G - 1:
                    nc.vector.tensor_tensor(
                        accMax[:, :], pb[:, :], accMax[:, :], Alu.max)
                    nc.vector.tensor_tensor(
                        accMin[:, :], pb[:, :], accMin[:, :], Alu.min)

        # merge the sub-accumulators G -> G/2 -> 1, then clamp at +-U
        tmpx = accp.tile([128, 2 * FD], F16)
        tmpn = accp.tile([128, 2 * FD], F16)
        nc.vector.tensor_tensor(
            tmpx[:, :], accMax[:, : 2 * FD], accMax[:, 2 * FD :], Alu.max)
        nc.vector.tensor_tensor(
            tmpn[:, :], accMin[:, : 2 * FD], accMin[:, 2 * FD :], Alu.min)
        Mx = accp.tile([128, FD], F16)
        Mn = accp.tile([128, FD], F16)
        nc.vector.tensor_tensor(Mx[:, :], tmpx[:, :FD], tmpx[:, FD:], Alu.max)
        nc.vector.tensor_tensor(Mn[:, :], tmpn[:, :FD], tmpn[:, FD:], Alu.min)
        nc.vector.tensor_scalar(
            out=Mx[:, :], in0=Mx[:, :],
            scalar1=xbf[:, UCOL : UCOL + 2].bitcast(F32), scalar2=None,
            op0=Alu.max)
        nc.vector.tensor_scalar(
            out=Mn[:, :], in0=Mn[:, :],
            scalar1=xbf[:, UCOL + 2 : UCOL + 4].bitcast(F32), scalar2=None,
            op0=Alu.min)

        # Combine with channels on partitions:
        #   pt[64, pos] = sum_q M1[q,c]*(Mx+Mn)[q,pos] = (mA1-mA2)+(aMin1-aMin2)
        # Activation stages PSUM->SBUF adding the bias; one 64-line DMA out.
        pt = tps.tile([128, FD], F32)
        for s, e in ((0, 512), (512, FD)):
            nc.tensor.matmul(pt[:COUT, s:e], lhsT=M1_sb[:, :], rhs=Mx[:, s:e],
                             start=True, stop=False)
            nc.tensor.matmul(pt[:COUT, s:e], lhsT=M1_sb[:, :], rhs=Mn[:, s:e],
                             start=False, stop=True)
        y32 = tsb.tile([COUT, FD], F32)
        nc.scalar.activation(
            out=y32[:, :], in_=pt[:COUT, :],
            func=mybir.ActivationFunctionType.Identity,
            bias=BC_sb[:, 0:1], scale=1.0)
        nc.sync.dma_start(out=Y[:, :], in_=y32[:, :])
    nc.finalize()
    return nc


def _host_prep(x, k1, k2, bias):
    x = np.ascontiguousarray(np.asarray(x, dtype=np.float32))
    K1 = np.exp(np.asarray(k1, np.float32).reshape(3, 3, C, COUT))
    K2 = np.exp(np.asarray(k2, np.float32).reshape(3, 3, C, COUT))
    # S[q, ci, t=i*3+j]: q<64 -> K1[i,j,ci,q];  q>=64 -> K2[i,j,ci,q-64]
    S1 = K1.transpose(3, 2, 0, 1).reshape(COUT, C, 9)
    S2 = K2.transpose(3, 2, 0, 1).reshape(COUT, C, 9)
    S = np.concatenate([S1, S2], axis=0).astype(np.float32)   # [128, C, 9]
    U1 = 0.1 * K1.reshape(9 * C, COUT).max(axis=0)
    U2_ = 0.1 * K2.reshape(9 * C, COUT).max(axis=0)
    U = np.concatenate([U1, U2_]).astype(np.float32)          # [128]
    M1 = np.vstack([np.eye(COUT, dtype=np.float16), -np.eye(COUT, dtype=np.float16)])
    BCc = np.asarray(bias, np.float32).reshape(COUT, 1)
    shared = dict(M1=np.ascontiguousarray(M1), BCc=np.ascontiguousarray(BCc))
    in_maps = []
    for n in range(N_CORES):
        rows = np.zeros((C, XLEN), np.float16)
        xr = x[n].reshape(NPIX, C).T.astype(np.float16)       # [C, 1024]
        rows[:, :NPIX] = xr
        rows[:, ROWL : ROWL + NPIX - 1] = xr[:, 1:]
        xb = np.broadcast_to(rows[:, None, :], (C, 128, XLEN)).copy()
        xb[:, :, SCOL:UCOL].view(np.float32)[:] = S.transpose(1, 0, 2)
        xb[:, :, UCOL : UCOL + 2].view(np.float32)[:, :, 0] = U[None, :]
        xb[:, :, UCOL + 2 : UCOL + 4].view(np.float32)[:, :, 0] = -U[None, :]
        in_maps.append({"XB": xb.reshape(C * 128, XLEN), **shared})
    return in_maps


def kernel(x, k1, k2, bias):
    global last_results
    if "nc" not in _cache:
        _cache["nc"] = _build_module()
    nc = _cache["nc"]
    in_maps = _host_prep(x, k1, k2, bias)
    trace = bool(int(os.environ.get("KTRACE", "0")))
    if trace:
        _ensure_axon_ntff_hook()
    res = run_bass_kernel_spmd(
        nc, in_maps, core_ids=list(range(N_CORES)), trace=trace,
    )
    last_results = res
    y = np.stack([r["Y"].reshape(COUT, HO, WO).transpose(1, 2, 0)
                  for r in res.results], axis=0)
    return np.ascontiguousarray(y, np.float32)


# revision 12
# speedup vs baseline: 2.0117x; 1.0090x over previous
"""Bipolar morphological conv2d kernel for Trainium2 (8 NeuronCores).

Math: reference computes, per output position and out-channel c,
    y = m(lp1,K1) - m(lp1,K2) - m(lp2,K1) + m(lp2,K2) + bias
with m(logp, k)[c] = exp(max_p(logp_p + k_pc)), lp1 = log(max(patch, .1)),
lp2 = log(max(-patch, .1)).

Since exp is monotone, m(lp1,K)[c] = max(U_c, max_p(x_p*K_pc)) and
m(lp2,K)[c] = max(U_c, -min_p(x_p*K_pc)) with K = exp(k) > 0 and
U_c = .1*max_p K_pc (the clamp folds into a per-channel constant).  So the
whole op needs ONE product set per kernel, max- AND min-reduced over taps:
    y = (mA1 - mA2) + (aMin1 - aMin2) + bias
with mA_k = max(U_k, max_p x_p*K_k), aMin_k = min(-U_k, min_p x_p*K_k).

Device strategy (data-parallel, one batch image per core):
  - partitions = 128 = [64 out-channels of K1 | 64 out-channels of K2]
  - free dim = 900 output positions as [30 rows, 30 cols] windows (row
    stride 32) into a per-ci broadcast row; host pre-replicates the rows
    across partitions in DRAM (fp16, even+odd parity copies so every tap
    window is 4B aligned).  The per-(tap,ci) kernel scalars and the U
    clamps ride along as trailing columns of the same tile, so one DMA
    per ci delivers everything and nothing else gates the first product.
  - products on the Activation engine (Copy with per-partition scale;
    Pool rejects TensorScalarPtr/TensorTensor at codegen), grouped 4 taps
    per buffer; the seed group and a small sliver run on DVE to balance.
  - folds on DVE: per group one tensor_tensor max + one min over
    [128, 4*900] fp16 (2x_1p mode). U clamp applied once post-merge.
  - combine: two accumulating PE matmuls lhsT=[I;-I] turn the merged accs
    into (col_K1 - col_K2) sums with channels on PARTITIONS ([64, 900]
    PSUM), Activation adds the bias while staging PSUM->SBUF, and one
    64-descriptor DMA writes Y [64, 900] (host transposes).
"""

import os
from contextlib import ExitStack

import numpy as np

import concourse.bass as bass
import concourse.mybir as mybir
from concourse import bacc
import concourse.tile as tile
from concourse.bass_utils import run_bass_kernel_spmd

N_CORES = 8
H = W = C = 32
COUT = 64
HO = WO = 30
NPIX = H * W          # 1024
FD = HO * WO          # 900 output positions, accessed as [30, 30] windows
ROWL = 1026           # even-parity row length (1024 pixels + 2 pad)
SCOL = 2 * ROWL       # fp32 per-tap scalars, packed as fp16 slot pairs
UCOL = SCOL + 18      # [U, -U] fp32 columns (2 slots each)
XLEN = UCOL + 4       # row length in fp16 slots (4B aligned)
P = 288               # 3*3*32 patch size
G = 4                 # taps per product buffer / fold group

F32 = mybir.dt.float32
F16 = mybir.dt.float16
_cache: dict = {}
last_results = None


def _ensure_axon_ntff_hook():
    """The trimmed agent image lacks antenv.axon_hooks; recreate it so
    run_bass_kernel_spmd(trace=True) can capture NTFF profiles. No-op on
    failure (tracing then just degrades)."""
    import sys
    import types

    try:
        import antenv.axon_hooks  # noqa: F401
        return
    except ImportError:
        pass
    try:
        mod = types.ModuleType("antenv.axon_hooks")
        holder = [None]
        mod.set_axon_ntff_profile_hook = lambda h: holder.__setitem__(0, h)
        mod.get_axon_ntff_profile_hook = lambda: holder[0]
        sys.modules["antenv.axon_hooks"] = mod
        from trn_agent_boot.trn_boot import _ntff_profile_via_ctypes

        so = "/opt/axon/libaxon_pjrt.so"
        if os.path.exists(so):
            holder[0] = _ntff_profile_via_ctypes(so)
    except Exception:
        pass


def _build_module():
    nc = bacc.Bacc()
    Alu = mybir.AluOpType

    XB = nc.dram_tensor("XB", [C * 128, XLEN], F16, kind="ExternalInput")
    M1 = nc.dram_tensor("M1", [128, COUT], F16, kind="ExternalInput")
    BCc = nc.dram_tensor("BCc", [COUT, 1], F32, kind="ExternalInput")
    Y = nc.dram_tensor("Y", [COUT, FD], F32, kind="ExternalOutput")

    with tile.TileContext(nc) as tc, ExitStack() as ctx:
        const = ctx.enter_context(tc.tile_pool(name="const", bufs=1))
        xbp = ctx.enter_context(tc.tile_pool(name="xbp", bufs=5))
        pbp = ctx.enter_context(tc.tile_pool(name="pbp", bufs=6))
        accp = ctx.enter_context(tc.tile_pool(name="accp", bufs=1))
        tps = ctx.enter_context(tc.tile_pool(name="tps", bufs=1, space="PSUM"))
        tsb = ctx.enter_context(tc.tile_pool(name="tsb", bufs=1))

        # xb[0] gates the first products: issue it before everything else.
        xb0 = xbp.tile([128, XLEN], F16, tag="xb")
        nc.sync.dma_start(out=xb0[:, :], in_=XB[0:128, :])
        M1_sb = const.tile([128, COUT], F16)
        nc.gpsimd.dma_start(out=M1_sb[:, :], in_=M1[:, :])
        BC_sb = const.tile([COUT, 1], F32)
        nc.gpsimd.dma_start(out=BC_sb[:, :], in_=BCc[:, :])

        accMax = accp.tile([128, G * FD], F16)
        accMin = accp.tile([128, G * FD], F16)

        pb = None
        xbf = xb0  # tile holding the (identical) U columns, kept live
        for ci in range(C):
            if ci == 0:
                xb_sb = xb0
            else:
                xb_sb = xbp.tile([128, XLEN], F16, tag="xb")
                nc.sync.dma_start(
                    out=xb_sb[:, :], in_=XB[ci * 128 : (ci + 1) * 128, :])
                if ci == C - 1:
                    xbf = xb_sb
            for t in range(9):
                i, j = divmod(t, 3)
                base = (ROWL + i * W) if j == 1 else (i * W + j)
                win = xb_sb[:, base : base + HO * W].rearrange(
                    "q (a b) -> q a b", b=W)[:, :, :WO]
                k = ci * 9 + t
                sc = xb_sb[:, SCOL + 2 * t : SCOL + 2 * t + 2].bitcast(F32)
                g, slot = divmod(k, G)
                if g == 0:
                    # first group seeds accMax directly; accMin is copied
                    # from it once (below) instead of duplicating products
                    dst = accMax
                elif slot == 0:
                    pb = pbp.tile([128, G * FD], F16, tag="pb")
                    dst = pb
                else:
                    dst = pb
                out_view = dst[:, slot * FD : (slot + 1) * FD].rearrange(
                    "q (a b) -> q a b", a=HO)
                if g == 0 or k % 48 == 47:
                    # seed group + a sliver of products run on DVE: it is
                    # idle during ramp-up and slightly under Act's load
                    nc.vector.tensor_scalar(
                        out=out_view, in0=win, scalar1=sc, scalar2=None,
                        op0=Alu.mult)
                else:
                    nc.scalar.mul(out=out_view, in_=win, mul=sc)
                if g == 0 and slot == G - 1:
                    nc.vector.tensor_scalar(
                        out=accMin[:, :], in0=accMax[:, :], scalar1=0.0,
                        scalar2=None, op0=Alu.add)
                if g > 0 and slot == G - 1:
                    nc.vector.tensor_tensor(
                        accMax[:, :], pb[:, :], accMax[:, :], Alu.max)
                    nc.vector.tensor_tensor(
                        accMin[:, :], pb[:, :], accMin[:, :], Alu.min)

        # Tail, pipelined in two PSUM-bank-aligned column halves:
        # merge sub-accs G -> G/2 -> 1, clamp at +-U, PE-combine with
        # channels on partitions, Act bias-add staging PSUM->SBUF, DMA out.
        tmpx = accp.tile([128, 2 * FD], F16)
        tmpn = accp.tile([128, 2 * FD], F16)
        Mx = accp.tile([128, FD], F16)
        Mn = accp.tile([128, FD], F16)
        pt = tps.tile([128, FD], F32)
        y32 = tsb.tile([COUT, FD], F32)
        accMax4 = accMax[:, :].rearrange("q (u f) -> q u f", f=FD)
        accMin4 = accMin[:, :].rearrange("q (u f) -> q u f", f=FD)
        tmpx2 = tmpx[:, :].rearrange("q (u f) -> q u f", f=FD)
        tmpn2 = tmpn[:, :].rearrange("q (u f) -> q u f", f=FD)
        for s, e in ((0, 512), (512, FD)):
            nc.vector.tensor_tensor(
                tmpx2[:, :, s:e], accMax4[:, 0:2, s:e], accMax4[:, 2:4, s:e],
                Alu.max)
            nc.vector.tensor_tensor(
                Mx[:, s:e], tmpx[:, s:e], tmpx[:, FD + s : FD + e], Alu.max)
            nc.vector.tensor_scalar(
                out=Mx[:, s:e], in0=Mx[:, s:e],
                scalar1=xbf[:, UCOL : UCOL + 2].bitcast(F32), scalar2=None,
                op0=Alu.max)
            nc.vector.tensor_tensor(
                tmpn2[:, :, s:e], accMin4[:, 0:2, s:e], accMin4[:, 2:4, s:e],
                Alu.min)
            nc.vector.tensor_tensor(
                Mn[:, s:e], tmpn[:, s:e], tmpn[:, FD + s : FD + e], Alu.min)
            nc.vector.tensor_scalar(
                out=Mn[:, s:e], in0=Mn[:, s:e],
                scalar1=xbf[:, UCOL + 2 : UCOL + 4].bitcast(F32), scalar2=None,
                op0=Alu.min)
            nc.tensor.matmul(pt[:COUT, s:e], lhsT=M1_sb[:, :], rhs=Mx[:, s:e],
                             start=True, stop=False)
            nc.tensor.matmul(pt[:COUT, s:e], lhsT=M1_sb[:, :], rhs=Mn[:, s:e],
                             start=False, stop=True)
            nc.scalar.activation(
                out=y32[:, s:e], in_=pt[:COUT, s:e],
                func=mybir.ActivationFunctionType.Identity,
                bias=BC_sb[:, 0:1], scale=1.0)
            nc.sync.dma_start(out=Y[:, s:e], in_=y32[:, s:e])
    nc.finalize()
    return nc


def _host_prep(x, k1, k2, bias):
    x = np.ascontiguousarray(np.asarray(x, dtype=np.float32))
    K1 = np.exp(np.asarray(k1, np.float32).reshape(3, 3, C, COUT))
    K2 = np.exp(np.asarray(k2, np.float32).reshape(3, 3, C, COUT))
    # S[q, ci, t=i*3+j]: q<64 -> K1[i,j,ci,q];  q>=64 -> K2[i,j,ci,q-64]
    S1 = K1.transpose(3, 2, 0, 1).reshape(COUT, C, 9)
    S2 = K2.transpose(3, 2, 0, 1).reshape(COUT, C, 9)
    S = np.concatenate([S1, S2], axis=0).astype(np.float32)   # [128, C, 9]
    U1 = 0.1 * K1.reshape(9 * C, COUT).max(axis=0)
    U2_ = 0.1 * K2.reshape(9 * C, COUT).max(axis=0)
    U = np.concatenate([U1, U2_]).astype(np.float32)          # [128]
    M1 = np.vstack([np.eye(COUT, dtype=np.float16), -np.eye(COUT, dtype=np.float16)])
    BCc = np.asarray(bias, np.float32).reshape(COUT, 1)
    shared = dict(M1=np.ascontiguousarray(M1), BCc=np.ascontiguousarray(BCc))
    in_maps = []
    for n in range(N_CORES):
        rows = np.zeros((C, XLEN), np.float16)
        xr = x[n].reshape(NPIX, C).T.astype(np.float16)       # [C, 1024]
        rows[:, :NPIX] = xr
        rows[:, ROWL : ROWL + NPIX - 1] = xr[:, 1:]
        xb = np.broadcast_to(rows[:, None, :], (C, 128, XLEN)).copy()
        xb[:, :, SCOL:UCOL].view(np.float32)[:] = S.transpose(1, 0, 2)
        xb[:, :, UCOL : UCOL + 2].view(np.float32)[:, :, 0] = U[None, :]
        xb[:, :, UCOL + 2 : UCOL + 4].view(np.float32)[:, :, 0] = -U[None, :]
        in_maps.append({"XB": xb.reshape(C * 128, XLEN), **shared})
    return in_maps


def kernel(x, k1, k2, bias):
    global last_results
    if "nc" not in _cache:
        _cache["nc"] = _build_module()
    nc = _cache["nc"]
    in_maps = _host_prep(x, k1, k2, bias)
    trace = bool(int(os.environ.get("KTRACE", "0")))
    if trace:
        _ensure_axon_ntff_hook()
    res = run_bass_kernel_spmd(
        nc, in_maps, core_ids=list(range(N_CORES)), trace=trace,
    )
    last_results = res
    y = np.stack([r["Y"].reshape(COUT, HO, WO).transpose(1, 2, 0)
                  for r in res.results], axis=0)
    return np.ascontiguousarray(y, np.float32)


# revision 13
# speedup vs baseline: 2.0118x; 1.0001x over previous
"""Bipolar morphological conv2d kernel for Trainium2 (8 NeuronCores).

Math: reference computes, per output position and out-channel c,
    y = m(lp1,K1) - m(lp1,K2) - m(lp2,K1) + m(lp2,K2) + bias
with m(logp, k)[c] = exp(max_p(logp_p + k_pc)), lp1 = log(max(patch, .1)),
lp2 = log(max(-patch, .1)).

Since exp is monotone, m(lp1,K)[c] = max(U_c, max_p(x_p*K_pc)) and
m(lp2,K)[c] = max(U_c, -min_p(x_p*K_pc)) with K = exp(k) > 0 and
U_c = .1*max_p K_pc (the clamp folds into a per-channel constant).  So the
whole op needs ONE product set per kernel, max- AND min-reduced over taps:
    y = (mA1 - mA2) + (aMin1 - aMin2) + bias
with mA_k = max(U_k, max_p x_p*K_k), aMin_k = min(-U_k, min_p x_p*K_k).

Device strategy (data-parallel, one batch image per core):
  - partitions = 128 = [64 out-channels of K1 | 64 out-channels of K2]
  - free dim = 900 output positions as [30 rows, 30 cols] windows (row
    stride 32) into a per-ci broadcast row; host pre-replicates the rows
    across partitions in DRAM (fp16, even+odd parity copies so every tap
    window is 4B aligned).  The per-(tap,ci) kernel scalars and the U
    clamps ride along as trailing columns of the same tile, so one DMA
    per ci delivers everything and nothing else gates the first product.
  - products on the Activation engine (Copy with per-partition scale;
    Pool rejects TensorScalarPtr/TensorTensor at codegen), grouped 4 taps
    per buffer; the seed group and a small sliver run on DVE to balance.
  - folds on DVE: per group one tensor_tensor max + one min over
    [128, 4*900] fp16 (2x_1p mode). U clamp applied once post-merge.
  - tail, pipelined in two PSUM-bank-aligned column halves: merge the
    sub-accumulators, clamp at +-U, then accumulating PE matmuls with
    lhsT=[I;-I] turn the accs into (col_K1 - col_K2) sums with channels
    on PARTITIONS ([64, *] PSUM — 64 DMA descriptor lines instead of
    900), Activation adds the bias while staging PSUM->SBUF, DMA out
    Y [64, 900] (host transposes back).
"""

import os
from contextlib import ExitStack

import numpy as np

import concourse.bass as bass
import concourse.mybir as mybir
from concourse import bacc
import concourse.tile as tile
from concourse.bass_utils import run_bass_kernel_spmd

N_CORES = 8
H = W = C = 32
COUT = 64
HO = WO = 30
NPIX = H * W          # 1024
FD = HO * WO          # 900 output positions, accessed as [30, 30] windows
ROWL = 1026           # even-parity row length (1024 pixels + 2 pad)
SCOL = 2 * ROWL       # fp32 per-tap scalars, packed as fp16 slot pairs
UCOL = SCOL + 18      # [U, -U] fp32 columns (2 slots each)
XLEN = UCOL + 4       # row length in fp16 slots (4B aligned)
P = 288               # 3*3*32 patch size
G = 4                 # taps per product buffer / fold group

F32 = mybir.dt.float32
F16 = mybir.dt.float16
_cache: dict = {}
last_results = None


def _ensure_axon_ntff_hook():
    """The trimmed agent image lacks antenv.axon_hooks; recreate it so
    run_bass_kernel_spmd(trace=True) can capture NTFF profiles. No-op on
    failure (tracing then just degrades)."""
    import sys
    import types

    try:
        import antenv.axon_hooks  # noqa: F401
        return
    except ImportError:
        pass
    try:
        mod = types.ModuleType("antenv.axon_hooks")
        holder = [None]
        mod.set_axon_ntff_profile_hook = lambda h: holder.__setitem__(0, h)
        mod.get_axon_ntff_profile_hook = lambda: holder[0]
        sys.modules["antenv.axon_hooks"] = mod
        from trn_agent_boot.trn_boot import _ntff_profile_via_ctypes

        so = "/opt/axon/libaxon_pjrt.so"
        if os.path.exists(so):
            holder[0] = _ntff_profile_via_ctypes(so)
    except Exception:
        pass


def _build_module():
    nc = bacc.Bacc()
    Alu = mybir.AluOpType

    XB = nc.dram_tensor("XB", [C * 128, XLEN], F16, kind="ExternalInput")
    M1 = nc.dram_tensor("M1", [128, COUT], F16, kind="ExternalInput")
    BCc = nc.dram_tensor("BCc", [COUT, 1], F32, kind="ExternalInput")
    Y = nc.dram_tensor("Y", [COUT, FD], F32, kind="ExternalOutput")

    with tile.TileContext(nc) as tc, ExitStack() as ctx:
        const = ctx.enter_context(tc.tile_pool(name="const", bufs=1))
        xbp = ctx.enter_context(tc.tile_pool(name="xbp", bufs=5))
        pbp = ctx.enter_context(tc.tile_pool(name="pbp", bufs=6))
        accp = ctx.enter_context(tc.tile_pool(name="accp", bufs=1))
        tps = ctx.enter_context(tc.tile_pool(name="tps", bufs=1, space="PSUM"))
        tsb = ctx.enter_context(tc.tile_pool(name="tsb", bufs=1))

        # xb[0] gates the first products: issue it before everything else.
        xb0 = xbp.tile([128, XLEN], F16, tag="xb")
        nc.sync.dma_start(out=xb0[:, :], in_=XB[0:128, :])
        M1_sb = const.tile([128, COUT], F16)
        nc.gpsimd.dma_start(out=M1_sb[:, :], in_=M1[:, :])
        BC_sb = const.tile([COUT, 1], F32)
        nc.gpsimd.dma_start(out=BC_sb[:, :], in_=BCc[:, :])

        accMax = accp.tile([128, G * FD], F16)
        accMin = accp.tile([128, G * FD], F16)

        pb = None
        xbf = xb0  # tile holding the (identical) U columns, kept live
        for ci in range(C):
            if ci == 0:
                xb_sb = xb0
            else:
                xb_sb = xbp.tile([128, XLEN], F16, tag="xb")
                nc.sync.dma_start(
                    out=xb_sb[:, :], in_=XB[ci * 128 : (ci + 1) * 128, :])
                if ci == C - 1:
                    xbf = xb_sb
            for t in range(9):
                i, j = divmod(t, 3)
                base = (ROWL + i * W) if j == 1 else (i * W + j)
                win = xb_sb[:, base : base + HO * W].rearrange(
                    "q (a b) -> q a b", b=W)[:, :, :WO]
                k = ci * 9 + t
                sc = xb_sb[:, SCOL + 2 * t : SCOL + 2 * t + 2].bitcast(F32)
                g, slot = divmod(k, G)
                if g == 0:
                    # first group seeds accMax directly; accMin is copied
                    # from it once (below) instead of duplicating products
                    dst = accMax
                elif slot == 0:
                    pb = pbp.tile([128, G * FD], F16, tag="pb")
                    dst = pb
                else:
                    dst = pb
                out_view = dst[:, slot * FD : (slot + 1) * FD].rearrange(
                    "q (a b) -> q a b", a=HO)
                if g == 0 or k % 48 == 47:
                    # seed group + a sliver of products run on DVE: it is
                    # idle during ramp-up and slightly under Act's load
                    nc.vector.tensor_scalar(
                        out=out_view, in0=win, scalar1=sc, scalar2=None,
                        op0=Alu.mult)
                else:
                    nc.scalar.mul(out=out_view, in_=win, mul=sc)
                if g == 0 and slot == G - 1:
                    nc.vector.tensor_scalar(
                        out=accMin[:, :], in0=accMax[:, :], scalar1=0.0,
                        scalar2=None, op0=Alu.add)
                if g > 0 and slot == G - 1:
                    nc.vector.tensor_tensor(
                        accMax[:, :], pb[:, :], accMax[:, :], Alu.max)
                    nc.vector.tensor_tensor(
                        accMin[:, :], pb[:, :], accMin[:, :], Alu.min)

        # Tail, pipelined in two PSUM-bank-aligned column halves:
        # merge sub-accs G -> G/2 -> 1, clamp at +-U, PE-combine with
        # channels on partitions, Act bias-add staging PSUM->SBUF, DMA out.
        tmpx = accp.tile([128, 2 * FD], F16)
        tmpn = accp.tile([128, 2 * FD], F16)
        Mx = accp.tile([128, FD], F16)
        Mn = accp.tile([128, FD], F16)
        pt = tps.tile([128, FD], F32)
        y32 = tsb.tile([COUT, FD], F32)
        accMax4 = accMax[:, :].rearrange("q (u f) -> q u f", f=FD)
        accMin4 = accMin[:, :].rearrange("q (u f) -> q u f", f=FD)
        tmpx2 = tmpx[:, :].rearrange("q (u f) -> q u f", f=FD)
        tmpn2 = tmpn[:, :].rearrange("q (u f) -> q u f", f=FD)
        for s, e in ((0, 512), (512, FD)):
            nc.vector.tensor_tensor(
                tmpx2[:, :, s:e], accMax4[:, 0:2, s:e], accMax4[:, 2:4, s:e],
                Alu.max)
            nc.vector.tensor_tensor(
                Mx[:, s:e], tmpx[:, s:e], tmpx[:, FD + s : FD + e], Alu.max)
            nc.vector.tensor_scalar(
                out=Mx[:, s:e], in0=Mx[:, s:e],
                scalar1=xbf[:, UCOL : UCOL + 2].bitcast(F32), scalar2=None,
                op0=Alu.max)
            nc.vector.tensor_tensor(
                tmpn2[:, :, s:e], accMin4[:, 0:2, s:e], accMin4[:, 2:4, s:e],
                Alu.min)
            nc.vector.tensor_tensor(
                Mn[:, s:e], tmpn[:, s:e], tmpn[:, FD + s : FD + e], Alu.min)
            nc.vector.tensor_scalar(
                out=Mn[:, s:e], in0=Mn[:, s:e],
                scalar1=xbf[:, UCOL + 2 : UCOL + 4].bitcast(F32), scalar2=None,
                op0=Alu.min)
            nc.tensor.matmul(pt[:COUT, s:e], lhsT=M1_sb[:, :], rhs=Mx[:, s:e],
                             start=True, stop=False)
            nc.tensor.matmul(pt[:COUT, s:e], lhsT=M1_sb[:, :], rhs=Mn[:, s:e],
                             start=False, stop=True)
            nc.scalar.activation(
                out=y32[:, s:e], in_=pt[:COUT, s:e],
                func=mybir.ActivationFunctionType.Identity,
                bias=BC_sb[:, 0:1], scale=1.0)
            nc.sync.dma_start(out=Y[:, s:e], in_=y32[:, s:e])
    nc.finalize()
    return nc


def _host_prep(x, k1, k2, bias):
    x = np.ascontiguousarray(np.asarray(x, dtype=np.float32))
    K1 = np.exp(np.asarray(k1, np.float32).reshape(3, 3, C, COUT))
    K2 = np.exp(np.asarray(k2, np.float32).reshape(3, 3, C, COUT))
    # S[q, ci, t=i*3+j]: q<64 -> K1[i,j,ci,q];  q>=64 -> K2[i,j,ci,q-64]
    S1 = K1.transpose(3, 2, 0, 1).reshape(COUT, C, 9)
    S2 = K2.transpose(3, 2, 0, 1).reshape(COUT, C, 9)
    S = np.concatenate([S1, S2], axis=0).astype(np.float32)   # [128, C, 9]
    U1 = 0.1 * K1.reshape(9 * C, COUT).max(axis=0)
    U2_ = 0.1 * K2.reshape(9 * C, COUT).max(axis=0)
    U = np.concatenate([U1, U2_]).astype(np.float32)          # [128]
    M1 = np.vstack([np.eye(COUT, dtype=np.float16), -np.eye(COUT, dtype=np.float16)])
    BCc = np.asarray(bias, np.float32).reshape(COUT, 1)
    shared = dict(M1=np.ascontiguousarray(M1), BCc=np.ascontiguousarray(BCc))
    in_maps = []
    for n in range(N_CORES):
        rows = np.zeros((C, XLEN), np.float16)
        xr = x[n].reshape(NPIX, C).T.astype(np.float16)       # [C, 1024]
        rows[:, :NPIX] = xr
        rows[:, ROWL : ROWL + NPIX - 1] = xr[:, 1:]
        xb = np.broadcast_to(rows[:, None, :], (C, 128, XLEN)).copy()
        xb[:, :, SCOL:UCOL].view(np.float32)[:] = S.transpose(1, 0, 2)
        xb[:, :, UCOL : UCOL + 2].view(np.float32)[:, :, 0] = U[None, :]
        xb[:, :, UCOL + 2 : UCOL + 4].view(np.float32)[:, :, 0] = -U[None, :]
        in_maps.append({"XB": xb.reshape(C * 128, XLEN), **shared})
    return in_maps


def kernel(x, k1, k2, bias):
    global last_results
    if "nc" not in _cache:
        _cache["nc"] = _build_module()
    nc = _cache["nc"]
    in_maps = _host_prep(x, k1, k2, bias)
    trace = bool(int(os.environ.get("KTRACE", "0")))
    if trace:
        _ensure_axon_ntff_hook()
    res = run_bass_kernel_spmd(
        nc, in_maps, core_ids=list(range(N_CORES)), trace=trace,
    )
    last_results = res
    y = np.stack([r["Y"].reshape(COUT, HO, WO).transpose(1, 2, 0)
                  for r in res.results], axis=0)
    return np.ascontiguousarray(y, np.float32)


# revision 15
# speedup vs baseline: 2.0125x; 1.0003x over previous
"""Bipolar morphological conv2d kernel for Trainium2 (8 NeuronCores).

Math: reference computes, per output position and out-channel c,
    y = m(lp1,K1) - m(lp1,K2) - m(lp2,K1) + m(lp2,K2) + bias
with m(logp, k)[c] = exp(max_p(logp_p + k_pc)), lp1 = log(max(patch, .1)),
lp2 = log(max(-patch, .1)).

Since exp is monotone, m(lp1,K)[c] = max(U_c, max_p(x_p*K_pc)) and
m(lp2,K)[c] = max(U_c, -min_p(x_p*K_pc)) with K = exp(k) > 0 and
U_c = .1*max_p K_pc (the clamp folds into a per-channel constant).  So the
whole op needs ONE product set per kernel, max- AND min-reduced over taps:
    y = (mA1 - mA2) + (aMin1 - aMin2) + bias
with mA_k = max(U_k, max_p x_p*K_k), aMin_k = min(-U_k, min_p x_p*K_k).

Device strategy (data-parallel, one batch image per core):
  - partitions = 128 = [64 out-channels of K1 | 64 out-channels of K2]
  - free dim = 900 output positions as [30 rows, 30 cols] windows (row
    stride 32) into a per-ci broadcast row; host pre-replicates the rows
    across partitions in DRAM (fp16, even+odd parity copies so every tap
    window is 4B aligned).  The per-(tap,ci) kernel scalars and the U
    clamps ride along as trailing columns of the same tile, so one DMA
    per ci delivers everything and nothing else gates the first product.
  - products on the Activation engine (Copy with per-partition scale;
    Pool rejects TensorScalarPtr/TensorTensor at codegen), grouped 4 taps
    per buffer; the seed group and a small sliver run on DVE to balance.
  - folds on DVE: per group one tensor_tensor max + one min over
    [128, 4*900] fp16 (2x_1p mode). U clamp applied once post-merge.
  - tail, pipelined in two PSUM-bank-aligned column halves: merge the
    sub-accumulators, clamp at +-U, then accumulating PE matmuls with
    lhsT=[I;-I] turn the accs into (col_K1 - col_K2) sums with channels
    on PARTITIONS ([64, *] PSUM — 64 DMA descriptor lines instead of
    900), Activation adds the bias while staging PSUM->SBUF, DMA out
    Y [64, 900] (host transposes back).
"""

import os
from contextlib import ExitStack

import numpy as np

import concourse.bass as bass
import concourse.mybir as mybir
from concourse import bacc
import concourse.tile as tile
from concourse.bass_utils import run_bass_kernel_spmd

N_CORES = 8
H = W = C = 32
COUT = 64
HO = WO = 30
NPIX = H * W          # 1024
FD = HO * WO          # 900 output positions, accessed as [30, 30] windows
ROWL = 1026           # even-parity row length (1024 pixels + 2 pad)
SCOL = 2 * ROWL       # fp32 per-tap scalars, packed as fp16 slot pairs
UCOL = SCOL + 18      # [U, -U] fp32 columns (2 slots each)
XLEN = UCOL + 4       # row length in fp16 slots (4B aligned)
P = 288               # 3*3*32 patch size
G = 4                 # taps per product buffer / fold group

F32 = mybir.dt.float32
F16 = mybir.dt.float16
_cache: dict = {}
last_results = None


def _ensure_axon_ntff_hook():
    """The trimmed agent image lacks antenv.axon_hooks; recreate it so
    run_bass_kernel_spmd(trace=True) can capture NTFF profiles. No-op on
    failure (tracing then just degrades)."""
    import sys
    import types

    try:
        import antenv.axon_hooks  # noqa: F401
        return
    except ImportError:
        pass
    try:
        mod = types.ModuleType("antenv.axon_hooks")
        holder = [None]
        mod.set_axon_ntff_profile_hook = lambda h: holder.__setitem__(0, h)
        mod.get_axon_ntff_profile_hook = lambda: holder[0]
        sys.modules["antenv.axon_hooks"] = mod
        from trn_agent_boot.trn_boot import _ntff_profile_via_ctypes

        so = "/opt/axon/libaxon_pjrt.so"
        if os.path.exists(so):
            holder[0] = _ntff_profile_via_ctypes(so)
    except Exception:
        pass


def _build_module():
    nc = bacc.Bacc()
    Alu = mybir.AluOpType

    XB = nc.dram_tensor("XB", [C * 128, XLEN], F16, kind="ExternalInput")
    M1 = nc.dram_tensor("M1", [128, COUT], F16, kind="ExternalInput")
    BCc = nc.dram_tensor("BCc", [COUT, 1], F32, kind="ExternalInput")
    Y = nc.dram_tensor("Y", [COUT, FD], F32, kind="ExternalOutput")

    with tile.TileContext(nc) as tc, ExitStack() as ctx:
        const = ctx.enter_context(tc.tile_pool(name="const", bufs=1))
        xbp = ctx.enter_context(tc.tile_pool(name="xbp", bufs=6))
        pbp = ctx.enter_context(tc.tile_pool(name="pbp", bufs=8))
        accp = ctx.enter_context(tc.tile_pool(name="accp", bufs=1))
        tps = ctx.enter_context(tc.tile_pool(name="tps", bufs=1, space="PSUM"))
        tsb = ctx.enter_context(tc.tile_pool(name="tsb", bufs=1))

        # xb[0] gates the first products: issue it before everything else.
        xb0 = xbp.tile([128, XLEN], F16, tag="xb")
        nc.sync.dma_start(out=xb0[:, :], in_=XB[0:128, :])
        M1_sb = const.tile([128, COUT], F16)
        nc.gpsimd.dma_start(out=M1_sb[:, :], in_=M1[:, :])
        BC_sb = const.tile([COUT, 1], F32)
        nc.gpsimd.dma_start(out=BC_sb[:, :], in_=BCc[:, :])

        accMax = accp.tile([128, G * FD], F16)
        accMin = accp.tile([128, G * FD], F16)

        pb = None
        xbf = xb0  # tile holding the (identical) U columns, kept live
        for ci in range(C):
            if ci == 0:
                xb_sb = xb0
            else:
                xb_sb = xbp.tile([128, XLEN], F16, tag="xb")
                nc.sync.dma_start(
                    out=xb_sb[:, :], in_=XB[ci * 128 : (ci + 1) * 128, :])
                if ci == C - 1:
                    xbf = xb_sb
            for t in range(9):
                i, j = divmod(t, 3)
                base = (ROWL + i * W) if j == 1 else (i * W + j)
                win = xb_sb[:, base : base + HO * W].rearrange(
                    "q (a b) -> q a b", b=W)[:, :, :WO]
                k = ci * 9 + t
                sc = xb_sb[:, SCOL + 2 * t : SCOL + 2 * t + 2].bitcast(F32)
                g, slot = divmod(k, G)
                if g == 0:
                    # first group seeds accMax directly; accMin is copied
                    # from it once (below) instead of duplicating products
                    dst = accMax
                elif slot == 0:
                    pb = pbp.tile([128, G * FD], F16, tag="pb")
                    dst = pb
                else:
                    dst = pb
                out_view = dst[:, slot * FD : (slot + 1) * FD].rearrange(
                    "q (a b) -> q a b", a=HO)
                if g == 0 or k % 48 == 47:
                    # seed group + a sliver of products run on DVE: it is
                    # idle during ramp-up and slightly under Act's load
                    nc.vector.tensor_scalar(
                        out=out_view, in0=win, scalar1=sc, scalar2=None,
                        op0=Alu.mult)
                else:
                    nc.scalar.mul(out=out_view, in_=win, mul=sc)
                if g == 0 and slot == G - 1:
                    nc.vector.tensor_scalar(
                        out=accMin[:, :], in0=accMax[:, :], scalar1=0.0,
                        scalar2=None, op0=Alu.add)
                if g > 0 and slot == G - 1:
                    nc.vector.tensor_tensor(
                        accMax[:, :], pb[:, :], accMax[:, :], Alu.max)
                    nc.vector.tensor_tensor(
                        accMin[:, :], pb[:, :], accMin[:, :], Alu.min)

        # Tail, pipelined in two PSUM-bank-aligned column halves:
        # merge sub-accs G -> G/2 -> 1, clamp at +-U, PE-combine with
        # channels on partitions, Act bias-add staging PSUM->SBUF, DMA out.
        tmpx = accp.tile([128, 2 * FD], F16)
        tmpn = accp.tile([128, 2 * FD], F16)
        Mx = accp.tile([128, FD], F16)
        Mn = accp.tile([128, FD], F16)
        pt = tps.tile([128, FD], F32)
        y32 = tsb.tile([COUT, FD], F32)
        accMax4 = accMax[:, :].rearrange("q (u f) -> q u f", f=FD)
        accMin4 = accMin[:, :].rearrange("q (u f) -> q u f", f=FD)
        tmpx2 = tmpx[:, :].rearrange("q (u f) -> q u f", f=FD)
        tmpn2 = tmpn[:, :].rearrange("q (u f) -> q u f", f=FD)
        for s, e in ((0, 512), (512, FD)):
            nc.vector.tensor_tensor(
                tmpx2[:, :, s:e], accMax4[:, 0:2, s:e], accMax4[:, 2:4, s:e],
                Alu.max)
            nc.vector.tensor_tensor(
                Mx[:, s:e], tmpx[:, s:e], tmpx[:, FD + s : FD + e], Alu.max)
            nc.vector.tensor_scalar(
                out=Mx[:, s:e], in0=Mx[:, s:e],
                scalar1=xbf[:, UCOL : UCOL + 2].bitcast(F32), scalar2=None,
                op0=Alu.max)
            nc.vector.tensor_tensor(
                tmpn2[:, :, s:e], accMin4[:, 0:2, s:e], accMin4[:, 2:4, s:e],
                Alu.min)
            nc.vector.tensor_tensor(
                Mn[:, s:e], tmpn[:, s:e], tmpn[:, FD + s : FD + e], Alu.min)
            nc.vector.tensor_scalar(
                out=Mn[:, s:e], in0=Mn[:, s:e],
                scalar1=xbf[:, UCOL + 2 : UCOL + 4].bitcast(F32), scalar2=None,
                op0=Alu.min)
            nc.tensor.matmul(pt[:COUT, s:e], lhsT=M1_sb[:, :], rhs=Mx[:, s:e],
                             start=True, stop=False)
            nc.tensor.matmul(pt[:COUT, s:e], lhsT=M1_sb[:, :], rhs=Mn[:, s:e],
                             start=False, stop=True)
            nc.scalar.activation(
                out=y32[:, s:e], in_=pt[:COUT, s:e],
                func=mybir.ActivationFunctionType.Identity,
                bias=BC_sb[:, 0:1], scale=1.0)
            nc.sync.dma_start(out=Y[:, s:e], in_=y32[:, s:e])
    nc.finalize()
    return nc


def _host_prep(x, k1, k2, bias):
    x = np.ascontiguousarray(np.asarray(x, dtype=np.float32))
    K1 = np.exp(np.asarray(k1, np.float32).reshape(3, 3, C, COUT))
    K2 = np.exp(np.asarray(k2, np.float32).reshape(3, 3, C, COUT))
    # S[q, ci, t=i*3+j]: q<64 -> K1[i,j,ci,q];  q>=64 -> K2[i,j,ci,q-64]
    S1 = K1.transpose(3, 2, 0, 1).reshape(COUT, C, 9)
    S2 = K2.transpose(3, 2, 0, 1).reshape(COUT, C, 9)
    S = np.concatenate([S1, S2], axis=0).astype(np.float32)   # [128, C, 9]
    U1 = 0.1 * K1.reshape(9 * C, COUT).max(axis=0)
    U2_ = 0.1 * K2.reshape(9 * C, COUT).max(axis=0)
    U = np.concatenate([U1, U2_]).astype(np.float32)          # [128]
    M1 = np.vstack([np.eye(COUT, dtype=np.float16), -np.eye(COUT, dtype=np.float16)])
    BCc = np.asarray(bias, np.float32).reshape(COUT, 1)
    shared = dict(M1=np.ascontiguousarray(M1), BCc=np.ascontiguousarray(BCc))
    in_maps = []
    for n in range(N_CORES):
        rows = np.zeros((C, XLEN), np.float16)
        xr = x[n].reshape(NPIX, C).T.astype(np.float16)       # [C, 1024]
        rows[:, :NPIX] = xr
        rows[:, ROWL : ROWL + NPIX - 1] = xr[:, 1:]
        xb = np.broadcast_to(rows[:, None, :], (C, 128, XLEN)).copy()
        xb[:, :, SCOL:UCOL].view(np.float32)[:] = S.transpose(1, 0, 2)
        xb[:, :, UCOL : UCOL + 2].view(np.float32)[:, :, 0] = U[None, :]
        xb[:, :, UCOL + 2 : UCOL + 4].view(np.float32)[:, :, 0] = -U[None, :]
        in_maps.append({"XB": xb.reshape(C * 128, XLEN), **shared})
    return in_maps


def kernel(x, k1, k2, bias):
    global last_results
    if "nc" not in _cache:
        _cache["nc"] = _build_module()
    nc = _cache["nc"]
    in_maps = _host_prep(x, k1, k2, bias)
    trace = bool(int(os.environ.get("KTRACE", "0")))
    if trace:
        _ensure_axon_ntff_hook()
    res = run_bass_kernel_spmd(
        nc, in_maps, core_ids=list(range(N_CORES)), trace=trace,
    )
    last_results = res
    y = np.stack([r["Y"].reshape(COUT, HO, WO).transpose(1, 2, 0)
                  for r in res.results], axis=0)
    return np.ascontiguousarray(y, np.float32)


# revision 16
# speedup vs baseline: 2.0139x; 1.0007x over previous
"""Bipolar morphological conv2d kernel for Trainium2 (8 NeuronCores).

Math: reference computes, per output position and out-channel c,
    y = m(lp1,K1) - m(lp1,K2) - m(lp2,K1) + m(lp2,K2) + bias
with m(logp, k)[c] = exp(max_p(logp_p + k_pc)), lp1 = log(max(patch, .1)),
lp2 = log(max(-patch, .1)).

Since exp is monotone, m(lp1,K)[c] = max(U_c, max_p(x_p*K_pc)) and
m(lp2,K)[c] = max(U_c, -min_p(x_p*K_pc)) with K = exp(k) > 0 and
U_c = .1*max_p K_pc (the clamp folds into a per-channel constant).  So the
whole op needs ONE product set per kernel, max- AND min-reduced over taps:
    y = (mA1 - mA2) + (aMin1 - aMin2) + bias
with mA_k = max(U_k, max_p x_p*K_k), aMin_k = min(-U_k, min_p x_p*K_k).

Device strategy (data-parallel, one batch image per core):
  - partitions = 128 = [64 out-channels of K1 | 64 out-channels of K2]
  - free dim = 900 output positions as [30 rows, 30 cols] windows (row
    stride 32) into a per-ci broadcast row; host pre-replicates the rows
    across partitions in DRAM (fp16, even+odd parity copies so every tap
    window is 4B aligned).  The per-(tap,ci) kernel scalars and the U
    clamps ride along as trailing columns of the same tile, so one DMA
    per ci delivers everything and nothing else gates the first product.
  - products on the Activation engine (Copy with per-partition scale;
    Pool rejects TensorScalarPtr/TensorTensor at codegen), grouped 4 taps
    per buffer; the seed group and a small sliver run on DVE to balance.
  - folds on DVE: per group one tensor_tensor max + one min over
    [128, 4*900] fp16 (2x_1p mode). U clamp applied once post-merge.
  - tail, pipelined in two PSUM-bank-aligned column halves: merge the
    sub-accumulators, clamp at +-U, then accumulating PE matmuls with
    lhsT=[I;-I] turn the accs into (col_K1 - col_K2) sums with channels
    on PARTITIONS ([64, *] PSUM — 64 DMA descriptor lines instead of
    900), Activation adds the bias while staging PSUM->SBUF, DMA out
    Y [64, 900] (host transposes back).
"""

import os
from contextlib import ExitStack

import numpy as np

import concourse.bass as bass
import concourse.mybir as mybir
from concourse import bacc
import concourse.tile as tile
from concourse.bass_utils import run_bass_kernel_spmd

N_CORES = 8
H = W = C = 32
COUT = 64
HO = WO = 30
NPIX = H * W          # 1024
FD = HO * WO          # 900 output positions, accessed as [30, 30] windows
ROWL = 1026           # even-parity row length (1024 pixels + 2 pad)
SCOL = 2 * ROWL       # fp32 per-tap scalars, packed as fp16 slot pairs
UCOL = SCOL + 18      # [U, -U] fp32 columns (2 slots each)
XLEN = UCOL + 4       # row length in fp16 slots (4B aligned)
P = 288               # 3*3*32 patch size
G = 4                 # taps per product buffer / fold group

F32 = mybir.dt.float32
F16 = mybir.dt.float16
_cache: dict = {}
last_results = None


def _ensure_axon_ntff_hook():
    """The trimmed agent image lacks antenv.axon_hooks; recreate it so
    run_bass_kernel_spmd(trace=True) can capture NTFF profiles. No-op on
    failure (tracing then just degrades)."""
    import sys
    import types

    try:
        import antenv.axon_hooks  # noqa: F401
        return
    except ImportError:
        pass
    try:
        mod = types.ModuleType("antenv.axon_hooks")
        holder = [None]
        mod.set_axon_ntff_profile_hook = lambda h: holder.__setitem__(0, h)
        mod.get_axon_ntff_profile_hook = lambda: holder[0]
        sys.modules["antenv.axon_hooks"] = mod
        from trn_agent_boot.trn_boot import _ntff_profile_via_ctypes

        so = "/opt/axon/libaxon_pjrt.so"
        if os.path.exists(so):
            holder[0] = _ntff_profile_via_ctypes(so)
    except Exception:
        pass


def _build_module():
    nc = bacc.Bacc()
    Alu = mybir.AluOpType

    XB = nc.dram_tensor("XB", [C * 128, XLEN], F16, kind="ExternalInput")
    M1 = nc.dram_tensor("M1", [128, COUT], F16, kind="ExternalInput")
    BCc = nc.dram_tensor("BCc", [COUT, 1], F32, kind="ExternalInput")
    Y = nc.dram_tensor("Y", [COUT, FD], F32, kind="ExternalOutput")

    with tile.TileContext(nc) as tc, ExitStack() as ctx:
        const = ctx.enter_context(tc.tile_pool(name="const", bufs=1))
        xbp = ctx.enter_context(tc.tile_pool(name="xbp", bufs=6))
        pbp = ctx.enter_context(tc.tile_pool(name="pbp", bufs=8))
        accp = ctx.enter_context(tc.tile_pool(name="accp", bufs=1))
        tps = ctx.enter_context(tc.tile_pool(name="tps", bufs=1, space="PSUM"))
        tsb = ctx.enter_context(tc.tile_pool(name="tsb", bufs=1))

        # xb[0] gates the first products: issue it before everything else.
        xb0 = xbp.tile([128, XLEN], F16, tag="xb")
        nc.sync.dma_start(out=xb0[:, :], in_=XB[0:128, :])
        M1_sb = const.tile([128, COUT], F16)
        nc.gpsimd.dma_start(out=M1_sb[:, :], in_=M1[:, :])
        BC_sb = const.tile([COUT, 1], F32)
        nc.gpsimd.dma_start(out=BC_sb[:, :], in_=BCc[:, :])

        accMax = accp.tile([128, G * FD], F16)
        accMin = accp.tile([128, G * FD], F16)

        pb = None
        xbf = xb0  # tile holding the (identical) U columns, kept live
        for ci in range(C):
            if ci == 0:
                xb_sb = xb0
            else:
                xb_sb = xbp.tile([128, XLEN], F16, tag="xb")
                nc.sync.dma_start(
                    out=xb_sb[:, :], in_=XB[ci * 128 : (ci + 1) * 128, :])
                if ci == C - 1:
                    xbf = xb_sb
            for t in range(9):
                i, j = divmod(t, 3)
                base = (ROWL + i * W) if j == 1 else (i * W + j)
                win = xb_sb[:, base : base + HO * W].rearrange(
                    "q (a b) -> q a b", b=W)[:, :, :WO]
                k = ci * 9 + t
                sc = xb_sb[:, SCOL + 2 * t : SCOL + 2 * t + 2].bitcast(F32)
                g, slot = divmod(k, G)
                if g == 0:
                    # first group seeds accMax directly; accMin is copied
                    # from it once (below) instead of duplicating products
                    dst = accMax
                elif slot == 0:
                    pb = pbp.tile([128, G * FD], F16, tag="pb")
                    dst = pb
                else:
                    dst = pb
                out_view = dst[:, slot * FD : (slot + 1) * FD].rearrange(
                    "q (a b) -> q a b", a=HO)
                if g == 0 or (k % 48 == 47 and g < 71):
                    # seed group + a sliver of products run on DVE: it is
                    # idle during ramp-up and slightly under Act's load
                    nc.vector.tensor_scalar(
                        out=out_view, in0=win, scalar1=sc, scalar2=None,
                        op0=Alu.mult)
                else:
                    nc.scalar.mul(out=out_view, in_=win, mul=sc)
                if g == 0 and slot == G - 1:
                    nc.vector.tensor_scalar(
                        out=accMin[:, :], in0=accMax[:, :], scalar1=0.0,
                        scalar2=None, op0=Alu.add)
                if 0 < g < 71 and slot == G - 1:
                    nc.vector.tensor_tensor(
                        accMax[:, :], pb[:, :], accMax[:, :], Alu.max)
                    nc.vector.tensor_tensor(
                        accMin[:, :], pb[:, :], accMin[:, :], Alu.min)

        # Tail. The accumulator merge tree (groups 0..70) runs EARLY --
        # hidden behind the Act products of the final group -- and the last
        # group's buffer is folded by a pairwise tree per column half, so
        # only ~5us of DVE work remains after the last product.
        tmpx = accp.tile([128, 2 * FD], F16)
        tmpn = accp.tile([128, 2 * FD], F16)
        M0x = accp.tile([128, FD], F16)
        M0n = accp.tile([128, FD], F16)
        nc.vector.tensor_tensor(
            tmpx[:, :], accMax[:, : 2 * FD], accMax[:, 2 * FD :], Alu.max)
        nc.vector.tensor_tensor(
            M0x[:, :], tmpx[:, :FD], tmpx[:, FD:], Alu.max)
        nc.vector.tensor_tensor(
            tmpn[:, :], accMin[:, : 2 * FD], accMin[:, 2 * FD :], Alu.min)
        nc.vector.tensor_tensor(
            M0n[:, :], tmpn[:, :FD], tmpn[:, FD:], Alu.min)

        Mx = accp.tile([128, FD], F16)
        Mn = accp.tile([128, FD], F16)
        t71x = accp.tile([128, 2 * FD], F16)
        t71n = accp.tile([128, 2 * FD], F16)
        pt = tps.tile([128, FD], F32)
        y32 = tsb.tile([COUT, FD], F32)
        pb4f = pb[:, :].rearrange("q (u f) -> q u f", f=FD)
        t71x2 = t71x[:, :].rearrange("q (u f) -> q u f", f=FD)
        t71n2 = t71n[:, :].rearrange("q (u f) -> q u f", f=FD)
        for s, e in ((0, 512), (512, FD)):
            nc.vector.tensor_tensor(
                t71x2[:, :, s:e], pb4f[:, 0:2, s:e], pb4f[:, 2:4, s:e], Alu.max)
            nc.vector.tensor_tensor(
                Mx[:, s:e], t71x[:, s:e], t71x[:, FD + s : FD + e], Alu.max)
            nc.vector.tensor_tensor(
                Mx[:, s:e], Mx[:, s:e], M0x[:, s:e], Alu.max)
            nc.vector.tensor_scalar(
                out=Mx[:, s:e], in0=Mx[:, s:e],
                scalar1=xbf[:, UCOL : UCOL + 2].bitcast(F32), scalar2=None,
                op0=Alu.max)
            nc.vector.tensor_tensor(
                t71n2[:, :, s:e], pb4f[:, 0:2, s:e], pb4f[:, 2:4, s:e], Alu.min)
            nc.vector.tensor_tensor(
                Mn[:, s:e], t71n[:, s:e], t71n[:, FD + s : FD + e], Alu.min)
            nc.vector.tensor_tensor(
                Mn[:, s:e], Mn[:, s:e], M0n[:, s:e], Alu.min)
            nc.vector.tensor_scalar(
                out=Mn[:, s:e], in0=Mn[:, s:e],
                scalar1=xbf[:, UCOL + 2 : UCOL + 4].bitcast(F32), scalar2=None,
                op0=Alu.min)
            nc.tensor.matmul(pt[:COUT, s:e], lhsT=M1_sb[:, :], rhs=Mx[:, s:e],
                             start=True, stop=False)
            nc.tensor.matmul(pt[:COUT, s:e], lhsT=M1_sb[:, :], rhs=Mn[:, s:e],
                             start=False, stop=True)
            nc.scalar.activation(
                out=y32[:, s:e], in_=pt[:COUT, s:e],
                func=mybir.ActivationFunctionType.Identity,
                bias=BC_sb[:, 0:1], scale=1.0)
            nc.sync.dma_start(out=Y[:, s:e], in_=y32[:, s:e])
    nc.finalize()
    return nc


def _host_prep(x, k1, k2, bias):
    x = np.ascontiguousarray(np.asarray(x, dtype=np.float32))
    K1 = np.exp(np.asarray(k1, np.float32).reshape(3, 3, C, COUT))
    K2 = np.exp(np.asarray(k2, np.float32).reshape(3, 3, C, COUT))
    # S[q, ci, t=i*3+j]: q<64 -> K1[i,j,ci,q];  q>=64 -> K2[i,j,ci,q-64]
    S1 = K1.transpose(3, 2, 0, 1).reshape(COUT, C, 9)
    S2 = K2.transpose(3, 2, 0, 1).reshape(COUT, C, 9)
    S = np.concatenate([S1, S2], axis=0).astype(np.float32)   # [128, C, 9]
    U1 = 0.1 * K1.reshape(9 * C, COUT).max(axis=0)
    U2_ = 0.1 * K2.reshape(9 * C, COUT).max(axis=0)
    U = np.concatenate([U1, U2_]).astype(np.float32)          # [128]
    M1 = np.vstack([np.eye(COUT, dtype=np.float16), -np.eye(COUT, dtype=np.float16)])
    BCc = np.asarray(bias, np.float32).reshape(COUT, 1)
    shared = dict(M1=np.ascontiguousarray(M1), BCc=np.ascontiguousarray(BCc))
    in_maps = []
    for n in range(N_CORES):
        rows = np.zeros((C, XLEN), np.float16)
        xr = x[n].reshape(NPIX, C).T.astype(np.float16)       # [C, 1024]
        rows[:, :NPIX] = xr
        rows[:, ROWL : ROWL + NPIX - 1] = xr[:, 1:]
        xb = np.broadcast_to(rows[:, None, :], (C, 128, XLEN)).copy()
        xb[:, :, SCOL:UCOL].view(np.float32)[:] = S.transpose(1, 0, 2)
        xb[:, :, UCOL : UCOL + 2].view(np.float32)[:, :, 0] = U[None, :]
        xb[:, :, UCOL + 2 : UCOL + 4].view(np.float32)[:, :, 0] = -U[None, :]
        in_maps.append({"XB": xb.reshape(C * 128, XLEN), **shared})
    return in_maps


def kernel(x, k1, k2, bias):
    global last_results
    if "nc" not in _cache:
        _cache["nc"] = _build_module()
    nc = _cache["nc"]
    in_maps = _host_prep(x, k1, k2, bias)
    trace = bool(int(os.environ.get("KTRACE", "0")))
    if trace:
        _ensure_axon_ntff_hook()
    res = run_bass_kernel_spmd(
        nc, in_maps, core_ids=list(range(N_CORES)), trace=trace,
    )
    last_results = res
    y = np.stack([r["Y"].reshape(COUT, HO, WO).transpose(1, 2, 0)
                  for r in res.results], axis=0)
    return np.ascontiguousarray(y, np.float32)
